# revision 13
# baseline (speedup 1.0000x reference)
"""2-layer LSTM decoder Bass/Tile kernel for TRN2 — fp8 DoubleRow,
split-batch software pipelining.

Per-core: B_local=128 batch rows, H=512, 64 steps, data-parallel over 8
cores (weights replicated, host pre-transposed/pre-quantized).

Layout "T": features on partitions, batch on the free dim. The 128 batch
rows are processed as TWO independent 64-row halves whose recurrences
interleave on the engines: while one half's tail chain (sigmoid -> c update
-> tanh -> h writes) runs on ACT/DVE, the other half's matmul streams run
on PE. This hides the serial per-layer tail latency that otherwise floors
the step time.

Numerics (validated vs reference in numpy sim, 6.2e-3):
  - i/f/o gate matmuls: fp8 e4m3 DoubleRow (K=256/instr, 0.5 cyc/row),
    weights x16; h quantized to fp8 per step.
  - g (tanh) gate matmuls: bf16 x bf16 (the tanh path dominates the error
    budget; sigmoid paths tolerate fp8 noise).
  - y feedback: in(t) = [y(t-1), x(t)], y(t-1) = s + bp with s = Wp h1.
    The y contribution to L0 gates is outer(wcol, s + bp + zb_y), emitted
    as one K=1 DoubleRow matmul per chunk against fp8 (hi, lo) rows of
    16*wcol, with s8 = fp8(s + bp + zb_y) from the proj result. Bias b1
    uses the same (hi, lo)-pair trick against a fp8 ones vector.
  - PSUM uniformly 16x; gate ACTs apply scale=1/16. Tail elementwise fp16
    (DVE 2x); c state fp16; h written as bf16 (g-rhs + proj) and fp8.

PSUM per (layer, half): one [128, 1024] f32 tile (2 banks); columns are
16 chunk-slots of 64 in order i(0-3) f(4-7) o(8-11) g(12-15), so one
768-wide sigmoid covers i|f|o and one ACT covers g. Start/stop are per
2KB bank: bank1 (o+g) closes first (tanh g fires early), bank0 (i+f)
closes last with the sigmoid + c-chain behind it.
"""

import numpy as np
from contextlib import ExitStack

import concourse.bass as bass
import concourse.bacc as bacc
import concourse.mybir as mybir
import concourse.tile as tile

F32 = mybir.dt.float32
F32R = mybir.dt.float32r
BF16 = mybir.dt.bfloat16
F16 = mybir.dt.float16
F8 = mybir.dt.float8e4
AF = mybir.ActivationFunctionType
OP = mybir.AluOpType
DRM = mybir.MatmulPerfMode.DoubleRow

P = 128           # batch rows per core
HB = 64           # batch rows per half
H = 512           # hidden
G = 2048          # 4*H gates
KC = 4            # K chunks of 128 across H
S = 64            # steps
EXO = 8
ZD = 16
SCL = 16.0        # weight scale baked into all gate-stream weights

# pytorch gate order by chunk: i: 0-3, f: 4-7, g: 8-11, o: 12-15
G_CHUNKS = (8, 9, 10, 11)


def slot(c):
    """column slot of chunk c in the [P, 1024] half-tile: i 0-3, f 4-7,
    o 8-11, g 12-15 (so i|f|o are contiguous for the merged sigmoid)."""
    return c if c < 8 else (c - 4 if c >= 12 else c + 4)


CH_B1 = [12, 13, 14, 15, 8, 9, 10, 11]  # bank1: o then g; stop at chunk 11
CH_B0 = [0, 1, 2, 3, 4, 5, 6, 7]        # bank0: i then f; stop at chunk 7
CH_ALL = CH_B1 + CH_B0

ROLES = {}  # instruction name -> role string (diagnostics only)
# ordered per-opcode role lists (diagnostics: trace slices pair up in order)
ORD = {"Matmult": [], "Activation": [], "TensorTensor": [], "TensorCopy": [],
       "Memset": []}


def _reg(kind, role):
    ORD[kind].append(role)


def _tag_role(inst, role):
    try:
        ROLES[inst.ins.name] = role
    except Exception:
        pass
    return inst


def build_kernel(nc: bass.Bass, steps: int):
    assert steps == S
    def di(name, shape, dt):
        return nc.dram_tensor(name, shape, dt, kind="ExternalInput").ap()

    # fp8 i/f/o weights, [p, k*G+g] = 16*W[g, k*128+p]
    w8h0 = di("w8h0", [P, KC * G], F8)
    w8i1 = di("w8i1", [P, KC * G], F8)
    w8h1 = di("w8h1", [P, KC * G], F8)
    # bf16 g-gate weights, [p, k*512+j] = 16*W[1024+j, k*128+p]
    wg0 = di("wg0", [P, KC * H], BF16)
    wgi1 = di("wgi1", [P, KC * H], BF16)
    wgh1 = di("wgh1", [P, KC * H], BF16)
    # L0 input weights: rows 0-7 exo cols x16, row 8 = 16*b0, row 9 = 16*wcol
    wa0 = di("wa0", [10, G], BF16)
    b1p = di("b1p", [1, 2 * G], F8)           # (hi, lo) of 16*b1
    wcp = di("wcp", [1, 2 * G], F8)           # (hi, lo) of 16*wcol
    wpT = di("wpT", [P, KC], F32)             # [p, k] = W_proj[0, k*128+p]
    bp = di("bp", [1, 1], F32)
    wzT = di("wzT", [ZD, 9], F32)             # W_z.T
    bz8 = di("bz8", [8, 1], F32)              # b_z[1:9]
    bz0 = di("bz0", [1, 1], F32)              # b_z[0]
    ones_row = di("ones_row", [1, S * P], BF16)
    zT = di("zT", [ZD, P], F32)               # z.T
    y0T = di("y0T", [1, P], F32)
    xfT = di("xfT", [EXO, S * P], BF16)       # [e, t*128+b] = x_future[b, t, e]
    # per-half states, [p, k*64+b] = h[b_global = k*128 + u*64 + b]
    h0b_d = [di(f"h0b{u}", [P, H // 2], BF16) for u in (0, 1)]
    h1b_d = [di(f"h1b{u}", [P, H // 2], BF16) for u in (0, 1)]
    h08_d = [di(f"h08{u}", [P, H // 2], F8) for u in (0, 1)]
    h18_d = [di(f"h18{u}", [P, H // 2], F8) for u in (0, 1)]
    c0_d = [di(f"c0T{u}", [P, H // 2], F16) for u in (0, 1)]
    c1_d = [di(f"c1T{u}", [P, H // 2], F16) for u in (0, 1)]
    out_d = nc.dram_tensor("out", [S, P], F32, kind="ExternalOutput").ap()

    with tile.TileContext(nc) as tc, ExitStack() as ctx:
        emit(ctx, tc, nc, locals())
    return nc


def emit(ctx, tc, nc, t_in):
    w8h0, w8i1, w8h1 = t_in["w8h0"], t_in["w8i1"], t_in["w8h1"]
    wg0, wgi1, wgh1 = t_in["wg0"], t_in["wgi1"], t_in["wgh1"]
    wa0, b1p, wcp, wpT, bp = (t_in["wa0"], t_in["b1p"], t_in["wcp"],
                              t_in["wpT"], t_in["bp"])
    wzT, bz8, bz0, zT = t_in["wzT"], t_in["bz8"], t_in["bz0"], t_in["zT"]
    ones_row, y0T, xfT = t_in["ones_row"], t_in["y0T"], t_in["xfT"]
    h0b_d, h1b_d, h08_d, h18_d = (t_in["h0b_d"], t_in["h1b_d"],
                                  t_in["h08_d"], t_in["h18_d"])
    c0_d, c1_d, out_d = t_in["c0_d"], t_in["c1_d"], t_in["out_d"]

    # ---- pools ----
    const = ctx.enter_context(tc.tile_pool(name="const", bufs=1))
    ldtmp = ctx.enter_context(tc.tile_pool(name="ldtmp", bufs=1))
    state = ctx.enter_context(tc.tile_pool(name="state", bufs=2))
    act = ctx.enter_context(tc.tile_pool(name="act", bufs=2))
    yo = ctx.enter_context(tc.tile_pool(name="yo", bufs=3))
    g0p = ctx.enter_context(tc.tile_pool(name="g0p", bufs=1, space="PSUM"))
    g1p = ctx.enter_context(tc.tile_pool(name="g1p", bufs=1, space="PSUM"))

    dma = nc.sync.dma_start
    adma = nc.scalar.dma_start
    gdma = nc.gpsimd.dma_start

    # ---- persistent SBUF ----
    W8h0 = const.tile([P, KC * G], F8, name="W8h0")
    W8i1 = const.tile([P, KC * G], F8, name="W8i1")
    W8h1 = const.tile([P, KC * G], F8, name="W8h1")
    Wg0 = const.tile([P, KC * H], BF16, name="Wg0")
    Wgi1 = const.tile([P, KC * H], BF16, name="Wgi1")
    Wgh1 = const.tile([P, KC * H], BF16, name="Wgh1")
    wa0_sb = const.tile([10, G], BF16, name="wa0_sb")
    b1p_sb = const.tile([1, 2 * G], F8, name="b1p_sb")
    wcp_sb = const.tile([1, 2 * G], F8, name="wcp_sb")
    ones8 = const.tile([1, 2 * HB], F8, name="ones8")
    wpT_sb = const.tile([P, KC], BF16, name="wpT_sb")
    inT = const.tile([10, S * P], BF16, name="inT")  # p0-7 exo, p8 ones, p9 y0
    zb8 = const.tile([8, P], BF16, name="zb8")     # z-bias for exo rows
    zby = const.tile([1, P], F32, name="zby")      # z-bias for the y slot
    bz8_sb = const.tile([8, 1], F32, name="bz8_sb")
    bz0_sb = const.tile([1, 1], F32, name="bz0_sb")
    bp_row = const.tile([1, P], F32, name="bp_row")
    zbybp = const.tile([1, P], F32, name="zbybp")  # zb_y + bp

    # 3D views used by DR matmuls
    W8h0v = W8h0.rearrange("p (k g) -> p k g", k=KC)
    W8i1v = W8i1.rearrange("p (k g) -> p k g", k=KC)
    W8h1v = W8h1.rearrange("p (k g) -> p k g", k=KC)
    b1v = b1p_sb.rearrange("o (two g) -> o two g", two=2)
    wcv = wcp_sb.rearrange("o (two g) -> o two g", two=2)
    on8v = ones8.rearrange("o (two b) -> o two b", two=2)

    # ---- init loads ----
    # SP queue: Wg0+W8h0 first (gate the step-0 A pass), then wa0
    dma(Wg0[:], wg0)
    dma(W8h0[:], w8h0)
    dma(wa0_sb[:], wa0)
    # gpsimd queue: z tensors (f32r cast loads), states, xfT, then L1 weights
    wzT_sb = ldtmp.tile([ZD, 9], F32R, name="wzT_sb", tag="wz")
    gdma(wzT_sb[:], wzT)
    zT_sb = ldtmp.tile([ZD, P], F32R, name="zT_sb", tag="zt")
    gdma(zT_sb[:], zT)
    def half_states(pfx, dt):
        return [state.tile([P, H // 2], dt, name=f"{pfx}{u}", tag=f"{pfx}{u}")
                for u in (0, 1)]

    h0b_c = half_states("h0b", BF16)
    h08_c = half_states("h08", F8)
    h1b_c = half_states("h1b", BF16)
    h18_c = half_states("h18", F8)
    c0_c = half_states("c0", F16)
    c1_c = half_states("c1", F16)
    for u in (0, 1):
        gdma(h0b_c[u][:], h0b_d[u])
        gdma(h08_c[u][:], h08_d[u])
    xfT_sb = ldtmp.tile([EXO, S * P], BF16, name="xfT_sb", tag="xf")
    gdma(xfT_sb[:], xfT)
    for u in (0, 1):
        gdma(c0_c[u][:], c0_d[u])
        gdma(h1b_c[u][:], h1b_d[u])
        gdma(h18_c[u][:], h18_d[u])
        gdma(c1_c[u][:], c1_d[u])
    gdma(Wgh1[:], wgh1)
    gdma(W8h1[:], w8h1)
    gdma(Wgi1[:], wgi1)
    gdma(W8i1[:], w8i1)
    # ACT queue: dummy sigmoid first so the table set loads immediately,
    # then the small admas that gate the zb ACTs
    dumm = ldtmp.tile([1, 1], F32, name="dumm", tag="dumm")
    _reg("Memset", "dumm")
    nc.vector.memset(dumm[:], 0.0)
    _reg("Activation", "dumm")
    nc.scalar.activation(dumm[:], dumm[:], AF.Sigmoid)
    adma(bz8_sb[:], bz8)
    adma(bz0_sb[:], bz0)
    y0T_sb = ldtmp.tile([1, P], F32, name="y0T_sb", tag="y0")
    adma(y0T_sb[:], y0T)
    adma(inT[8:9, :], ones_row)  # b0 ones row (host const)
    adma(b1p_sb[:], b1p)
    adma(wcp_sb[:], wcp)

    _reg("Memset", "ones8")
    nc.vector.memset(ones8[:], 1.0)

    # ---- z bias: zb8[8, P] = W_z[1:9] @ z.T + b_z[1:9]; zby = row 0 ----
    zb_ps = g0p.tile([P, 2 * H], F32, name="zbps", tag="u0")
    _reg("Matmult", "zb")
    nc.tensor.matmul(
        zb_ps[0:8, 0:P], wzT_sb[:, 1:9], zT_sb[:],
        start=True, stop=True,
    )
    _reg("Matmult", "zb")
    nc.tensor.matmul(
        zb_ps[0:1, P : 2 * P], wzT_sb[:, 0:1], zT_sb[:],
        start=True, stop=True,
    )
    _reg("Activation", "zb8")
    nc.scalar.activation(zb8[:], zb_ps[0:8, 0:P], AF.Identity, bias=bz8_sb[:])
    _reg("Activation", "zby")
    nc.scalar.activation(zby[:], zb_ps[0:1, P : 2 * P], AF.Identity, bias=bz0_sb[:])

    # rest of the ACT-queue loads (after the zb ACTs so they don't gate them)
    bp_sb = ldtmp.tile([1, 1], F32, name="bp_sb", tag="bp")
    adma(bp_sb[:], bp)
    wpf = ldtmp.tile([P, KC], F32, name="wpf", tag="wp")
    adma(wpf[:], wpT)
    _reg("TensorCopy", "wpT")
    nc.vector.tensor_copy(wpT_sb[:], wpf[:])  # f32 -> bf16

    # exo rows with z-bias baked in: write straight into inT rows 0-7
    SPLIT = 4
    x3 = xfT_sb.rearrange("e (t b) -> e t b", b=P)
    dst3 = inT[0:8, :].rearrange("e (t b) -> e t b", b=P)
    zb3a = zb8.unsqueeze(1).broadcast_to((EXO, SPLIT, P))
    zb3b = zb8.unsqueeze(1).broadcast_to((EXO, S - SPLIT, P))
    _reg("TensorTensor", "exo_a")
    nc.vector.tensor_tensor(dst3[:, 0:SPLIT], x3[:, 0:SPLIT], zb3a, op=OP.add)
    # y0 row (partition 9, engine-unwritable): staged base-0 then DMA'd.
    # Only block 0 is ever read (t=0 uses K=10; t>0 uses K=9).
    fr0 = ldtmp.tile([1, P], BF16, name="fr0", tag="fr0")
    _reg("TensorTensor", "fr0")
    nc.vector.tensor_tensor(fr0[:], y0T_sb[:], zby[:], op=OP.add)
    adma(inT[9:10, 0:P], fr0[:])
    _reg("TensorCopy", "bp_row")
    nc.vector.tensor_copy(bp_row[:], bp_sb[0:1, 0:1].broadcast_to((1, P)))
    _reg("TensorTensor", "zbybp")
    nc.vector.tensor_tensor(zbybp[:], zby[:], bp_row[:], op=OP.add)
    _reg("TensorTensor", "exo_b")
    nc.vector.tensor_tensor(dst3[:, SPLIT:S], x3[:, SPLIT:S], zb3b, op=OP.add)

    # =========================== STEP LOOP ===========================
    CUR = ["init"]

    def R(role):
        CUR[0] = role

    def mm(*a, **k):
        _reg("Matmult", CUR[0])
        return _tag_role(nc.tensor.matmul(*a, **k), CUR[0])

    def sact(*a, **k):
        _reg("Activation", CUR[0])
        return _tag_role(nc.scalar.activation(*a, **k), CUR[0])

    def vtt(*a, **k):
        _reg("TensorTensor", CUR[0])
        return _tag_role(nc.vector.tensor_tensor(*a, **k), CUR[0])

    def g_tile(pool, u, pfx):
        return pool.tile([P, 2 * H], F32, name=pfx + f"u{u}", tag=f"u{u}")

    def hv(t4):
        """[P, KC*HB]-layout half state -> [P, KC, HB] view."""
        return t4.rearrange("p (k b) -> p k b", b=HB)

    def bkey(c):
        return "b0" if slot(c) < 8 else "b1"

    def dst_of(gt, c):
        s = slot(c)
        return gt[:, s * HB : (s + 1) * HB]

    def hid_mms(gt, c, W8v, Wg, h8u, hbu, started, stop=False):
        """Hidden-side matmuls for one gate chunk of one half: fp8 DR for
        i/f/o, bf16 for g. One PSUM start per bank (zero region)."""
        dst = dst_of(gt, c)
        bk = bkey(c)
        if c in G_CHUNKS:
            j0 = (c - 8) * P
            hb3 = hv(hbu)
            for k in range(KC):
                mm(
                    dst, Wg[:, k * H + j0 : k * H + j0 + P],
                    hb3[:, k : k + 1, :],
                    start=not started.get(bk, False),
                    stop=(stop and k == KC - 1),
                )
                started[bk] = True
        else:
            h83 = hv(h8u)
            for k2 in range(2):
                mm(
                    dst, W8v[:, 2 * k2 : 2 * k2 + 2, c * P : (c + 1) * P],
                    h83[:, 2 * k2 : 2 * k2 + 2, :],
                    start=not started.get(bk, False),
                    stop=(stop and k2 == 1),
                    perf_mode=DRM,
                )
                started[bk] = True

    def emit_A(gt, h08u, h0bu, started):
        R("A")
        for c in CH_ALL:
            hid_mms(gt, c, W8h0v, Wg0, h08u, h0bu, started)

    def emit_B(gt, t, u, parts=None):
        """L0 input matmuls: K=10 at t=0 (y0 row), K=9 after (exo + ones).
        With parts (t=0 only): carries the bank stops."""
        K = 10 if t == 0 else 9
        rhs = inT[0:K, t * P + u * HB : t * P + u * HB + HB]
        if parts is None:
            R("B")
            for c in CH_ALL:
                mm(dst_of(gt, c), wa0_sb[0:K, c * P : (c + 1) * P], rhs,
                   start=False, stop=False)
        else:
            R("B")
            for c in CH_B1:
                mm(dst_of(gt, c), wa0_sb[0:K, c * P : (c + 1) * P], rhs,
                   start=False, stop=(c == 11))
            parts[0]()
            R("B")
            for c in CH_B0:
                mm(dst_of(gt, c), wa0_sb[0:K, c * P : (c + 1) * P], rhs,
                   start=False, stop=(c == 7))
            parts[1]()  # sg only; caller runs the rest via run_tails

    def emit_feed(gt, s8v, parts):
        """y-feedback: one K=1 DR outer product per chunk against the fp8
        (hi, lo) wcol rows; carries the bank stops; tail parts interleave."""
        R("feed")
        for c in CH_B1:
            mm(dst_of(gt, c), wcv[:, :, c * P : (c + 1) * P], s8v,
               start=False, stop=(c == 11), perf_mode=DRM)
        parts[0]()
        R("feed")
        for c in CH_B0:
            mm(dst_of(gt, c), wcv[:, :, c * P : (c + 1) * P], s8v,
               start=False, stop=(c == 7), perf_mode=DRM)
        parts[1]()

    def emit_C(gt, h18u, h1bu, started):
        """L1 h1-part matmuls + fp8 DR bias (start-side)."""
        for c in CH_ALL:
            R("C")
            hid_mms(gt, c, W8h1v, Wgh1, h18u, h1bu, started)
            R("Cb")
            mm(dst_of(gt, c), b1v[:, :, c * P : (c + 1) * P],
               on8v[:], start=False, stop=False, perf_mode=DRM)

    def emit_D(gt, h08u, h0bu, parts):
        """L1 h0-part matmuls: the bank-closing stops; tail parts
        interleave between bank groups."""
        R("D")
        for c in CH_B1:
            hid_mms(gt, c, W8i1v, Wgi1, h08u, h0bu, {bkey(c): True},
                    stop=(c == 11))
        parts[0]()
        R("D")
        for c in CH_B0:
            hid_mms(gt, c, W8i1v, Wgi1, h08u, h0bu, {bkey(c): True},
                    stop=(c == 7))
        parts[1]()

    def make_tail(gt, c_prev_u, u, c_n_u, h_b_u, h_8_u, htag, h8_first):
        """Tail for one (layer, half) as 5 staged callbacks (tg, sg, c-chain,
        tanh(c), h-writes). The caller interleaves stages across the two
        halves so neither half's pending op blocks the other's ready op on
        the in-order ACT/DVE queues."""
        sg = act.tile([P, 3 * H // 2], F16, name=f"sg{u}", tag=f"sg{u}")
        tgt = act.tile([P, H // 2], F16, name=f"tg{u}", tag=f"tg{u}")
        t1 = act.tile([P, H // 2], F16, name=f"t1{u}", tag=f"t1{u}")
        t2 = act.tile([P, H // 2], F16, name=f"t2{u}", tag=f"t2{u}")
        thc = act.tile([P, H // 2], F16, name=f"thc{u}", tag=f"thc{u}")
        cpv = hv(c_prev_u)
        cnv = hv(c_n_u)
        hbv = hv(h_b_u)
        h8v = hv(h_8_u)
        sg3 = sg.rearrange("p (s b) -> p s b", b=HB)
        sgi, sgf, sgo = sg3[:, 0:4], sg3[:, 4:8], sg3[:, 8:12]
        tg3 = tgt.rearrange("p (s b) -> p s b", b=HB)
        t13 = t1.rearrange("p (s b) -> p s b", b=HB)
        t23 = t2.rearrange("p (s b) -> p s b", b=HB)
        th3 = thc.rearrange("p (s b) -> p s b", b=HB)
        ISC = 1.0 / SCL

        def p_g():
            R(htag + f":tg{u}")
            sact(tg3[:], gt[:, 12 * HB : 16 * HB].rearrange(
                "p (s b) -> p s b", b=HB), AF.Tanh, scale=ISC)

        def p_sg():
            R(htag + f":sg{u}")
            sact(sg[:], gt[:, 0 : 12 * HB], AF.Sigmoid, scale=ISC)

        def p_chain():
            R(htag + f":cchain{u}")
            vtt(t23[:], sgf, cpv, op=OP.mult)
            vtt(t13[:], sgi, tg3[:], op=OP.mult)
            vtt(cnv, t13[:], t23[:], op=OP.add)

        def p_thc():
            R(htag + f":thc{u}")
            sact(th3[:], cnv, AF.Tanh)

        def p_hwr():
            R(htag + f":hwr{u}")
            outs = [h8v, hbv] if h8_first else [hbv, h8v]
            for dst in outs:
                vtt(dst, sgo, th3[:], op=OP.mult)

        return [p_g, p_sg, p_chain, p_thc, p_hwr]

    def run_tails(tails):
        """Interleave the two halves' later tail stages: ACT gets
        sg0, sg1, thc0, thc1 while DVE runs chain0, chain1, hwr0, hwr1."""
        tails[0][2](); tails[1][2]()
        tails[0][3](); tails[1][3]()
        tails[0][4](); tails[1][4]()

    # prologue: step-0 L0 gates for both halves (+tails via B stops), L1 start
    g0_cur = [g_tile(g0p, u, "g0") for u in (0, 1)]
    for u in (0, 1):
        emit_A(g0_cur[u], h08_c[u], h0b_c[u], {})
    c0_n = half_states("c0", F16)
    h0b_n = half_states("h0b", BF16)
    h08_n = half_states("h08", F8)
    tails = [make_tail(g0_cur[u], c0_c[u], u, c0_n[u], h0b_n[u], h08_n[u],
                       "h0", h8_first=True) for u in (0, 1)]
    for u in (0, 1):
        emit_B(g0_cur[u], 0, u, tails[u])
    run_tails(tails)
    c0_c, h0b_cur, h08_cur = c0_n, h0b_n, h08_n
    g1_cur = [g_tile(g1p, u, "g1") for u in (0, 1)]
    for u in (0, 1):
        emit_C(g1_cur[u], h18_c[u], h1b_c[u], {})

    for t in range(S):
        # ---- L1 gates close for t (D), tails interleaved, both halves ----
        c1_n = half_states("c1", F16)
        h1b_n = half_states("h1b", BF16)
        h18_n = half_states("h18", F8)
        tails1 = [make_tail(g1_cur[u], c1_c[u], u, c1_n[u], h1b_n[u],
                            h18_n[u], "h1", h8_first=False) for u in (0, 1)]
        for u in (0, 1):
            emit_D(g1_cur[u], h08_cur[u], h0b_cur[u], tails1[u])
        run_tails(tails1)
        c1_c = c1_n
        # ---- L0 gates for t+1: hidden + input (start-side) ----
        if t + 1 < S:
            g0_nxt = [g_tile(g0p, u, "g0") for u in (0, 1)]
            for u in (0, 1):
                st = {}
                emit_A(g0_nxt[u], h08_cur[u], h0b_cur[u], st)
                emit_B(g0_nxt[u], t + 1, u)
        # ---- proj + y outputs + s8 per half ----
        s8vs = []
        for u in (0, 1):
            u0, u1 = u * HB, u * HB + HB
            h1b3 = hv(h1b_n[u])
            pj = g1_cur[u][0:1, 11 * HB : 12 * HB]
            R("proj")
            for k in range(KC):
                mm(pj, wpT_sb[:, k : k + 1], h1b3[:, k : k + 1, :],
                   start=(k == 0), stop=(k == KC - 1))
            s8 = yo.tile([1, 2 * HB], F8, name=f"s8{u}", tag=f"s8{u}")
            s8v = s8.rearrange("o (two b) -> o two b", two=2)
            R("s8")
            vtt(s8v[:],
                pj.unsqueeze(1).broadcast_to((1, 2, HB)),
                zbybp[:, u0:u1].unsqueeze(1).broadcast_to((1, 2, HB)),
                op=OP.add)
            s8vs.append(s8v)
            yo_t = yo.tile([1, HB], F32, name=f"yout{u}", tag=f"yout{u}")
            R("y_out")
            vtt(yo_t[:], pj, bp_row[:, u0:u1], op=OP.add)
            dma(out_d[t : t + 1, u0:u1], yo_t[:])
        # ---- L0 closes for t+1 (feed), tails interleaved; L1 starts ----
        if t + 1 < S:
            c0_n = half_states("c0", F16)
            h0b_n = half_states("h0b", BF16)
            h08_n = half_states("h08", F8)
            tails0 = [make_tail(g0_nxt[u], c0_c[u], u, c0_n[u], h0b_n[u],
                                h08_n[u], "h0", h8_first=True) for u in (0, 1)]
            for u in (0, 1):
                emit_feed(g0_nxt[u], s8vs[u], tails0[u])
            run_tails(tails0)
            c0_c, h0b_cur, h08_cur = c0_n, h0b_n, h08_n
            g1_nxt = [g_tile(g1p, u, "g1") for u in (0, 1)]
            for u in (0, 1):
                emit_C(g1_nxt[u], h18_n[u], h1b_n[u], {})
            g0_cur, g1_cur = g0_nxt, g1_nxt


def make_nc(steps: int = S):
    nc = bacc.Bacc("TRN2", target_bir_lowering=False, debug=False)
    build_kernel(nc, steps)
    nc.compile()
    return nc


# ======================= host-side prep =======================

def _bf16(x):
    import ml_dtypes
    return np.ascontiguousarray(np.asarray(x, np.float32).astype(ml_dtypes.bfloat16))


def _f8(x):
    import ml_dtypes
    return np.ascontiguousarray(np.asarray(x, np.float32).astype(ml_dtypes.float8_e4m3fn))


def _f8pair(v):
    """f32 vector -> [1, 2*len] fp8 (hi row, lo row) of 16*v."""
    import ml_dtypes
    v = np.asarray(v, np.float32) * SCL
    hi = v.astype(ml_dtypes.float8_e4m3fn)
    lo = (v - hi.astype(np.float32)).astype(ml_dtypes.float8_e4m3fn)
    return np.ascontiguousarray(np.stack([hi, lo]).reshape(1, 2 * v.size))


def _f32(x):
    return np.ascontiguousarray(np.asarray(x, dtype=np.float32))


def _wT(W):
    """[G', H] f32 -> [128, KC*G'] with [p, k*G'+g] = W[g, k*128+p]."""
    W = np.asarray(W, dtype=np.float32)
    Gp = W.shape[0]
    return W.T.reshape(KC, P, Gp).transpose(1, 0, 2).reshape(P, KC * Gp)


def _hT4(h):
    """[B_loc, H] -> [128, H] with [p, k*128+b] = h[b, k*128+p]."""
    return h.T.reshape(KC, P, P).transpose(1, 0, 2).reshape(P, H)


def shard_inputs(inputs, steps: int):
    B = inputs["y0"].shape[0]
    nb = B // P
    Wih0 = np.asarray(inputs["W_ih0"], np.float32)
    Wp = np.asarray(inputs["W_proj"], np.float32)
    b0 = _f32(inputs["b_ih0"]) + _f32(inputs["b_hh0"])
    b1 = _f32(inputs["b_ih1"]) + _f32(inputs["b_hh1"])
    wa0 = np.concatenate(
        [
            Wih0[:, 1:9].T * SCL,          # exo rows
            b0.reshape(1, G) * SCL,        # ones-row bias
            Wih0[:, 0:1].T * SCL,          # y0 row (t=0 only)
        ],
        axis=0,
    )

    def wsplit(W):
        W = np.asarray(W, np.float32) * SCL
        return _f8(_wT(W)), _bf16(_wT(W[1024:1536]))

    w8h0, wg0 = wsplit(inputs["W_hh0"])
    w8i1, wgi1 = wsplit(inputs["W_ih1"])
    w8h1, wgh1 = wsplit(inputs["W_hh1"])
    rep = {
        "w8h0": w8h0, "wg0": wg0,
        "w8i1": w8i1, "wgi1": wgi1,
        "w8h1": w8h1, "wgh1": wgh1,
        "wa0": _bf16(wa0),
        "b1p": _f8pair(b1),
        "wcp": _f8pair(Wih0[:, 0]),
        "wpT": _f32(Wp).reshape(KC, P).T.copy(),
        "bp": _f32(inputs["b_proj"]).reshape(1, 1),
        "wzT": _f32(inputs["W_z"]).T.copy(),
        "bz8": _f32(inputs["b_z"]).reshape(9, 1)[1:9].copy(),
        "bz0": _f32(inputs["b_z"]).reshape(9, 1)[0:1].copy(),
        "ones_row": _bf16(np.ones((1, S * P), np.float32)),
    }
    maps = []
    for i in range(nb):
        s = slice(i * P, (i + 1) * P)
        x = np.asarray(inputs["x_future"], np.float32)[s, :steps]  # [P, S, E]
        h0b = _bf16(_hT4(np.asarray(inputs["h0"], np.float32)[0, s]))
        h1b = _bf16(_hT4(np.asarray(inputs["h0"], np.float32)[1, s]))
        c0h = _hT4(np.asarray(inputs["c0"], np.float32)[0, s]).astype(np.float16)
        c1h = _hT4(np.asarray(inputs["c0"], np.float32)[1, s]).astype(np.float16)

        def hsplit(full, u):
            """[P, 512] = [p, k*128+b] -> half [P, 256] = [p, k*64+b]"""
            v = np.asarray(full).reshape(P, KC, P)[:, :, u * HB : (u + 1) * HB]
            return np.ascontiguousarray(v.reshape(P, KC * HB))

        m = dict(rep)
        m.update(
            {
                "zT": _f32(inputs["z"][s]).T.copy(),
                "y0T": _f32(inputs["y0"][s]).reshape(1, P).copy(),
                "xfT": _bf16(x.transpose(2, 1, 0).reshape(EXO, steps * P)),
            }
        )
        for u in (0, 1):
            m[f"h0b{u}"] = hsplit(h0b, u)
            m[f"h1b{u}"] = hsplit(h1b, u)
            m[f"h08{u}"] = hsplit(_f8(h0b.astype(np.float32)), u)
            m[f"h18{u}"] = hsplit(_f8(h1b.astype(np.float32)), u)
            m[f"c0T{u}"] = hsplit(c0h, u)
            m[f"c1T{u}"] = hsplit(c1h, u)
        maps.append(m)
    return maps


def assemble_output(results, steps: int):
    outs = [
        np.ascontiguousarray(np.asarray(rm["out"]).T).reshape(P, steps, 1)
        for rm in results
    ]
    return np.concatenate(outs, axis=0)


# ======================= public entry point =======================
_NC_CACHE = {}


def _get_nc():
    if "nc" not in _NC_CACHE:
        _NC_CACHE["nc"] = make_nc(S)
    return _NC_CACHE["nc"]


STEPS = S
N_CORES = 8


def kernel(**inputs):
    """Full-input entry point: shards batch over 8 NeuronCores, runs the
    Bass LSTM-decoder kernel, reassembles [B, steps, 1] float32 output."""
    from concourse.bass_utils import run_bass_kernel_spmd

    steps = int(inputs.get("steps", STEPS))
    assert steps == STEPS, f"kernel compiled for {STEPS} steps, got {steps}"
    nc = _get_nc()
    maps = shard_inputs(inputs, STEPS)
    res = run_bass_kernel_spmd(nc, maps, list(range(N_CORES)))
    return assemble_output(res.results, STEPS).astype(np.float32)


# revision 14
# speedup vs baseline: 1.0550x; 1.0550x over previous
"""2-layer LSTM decoder Bass/Tile kernel for TRN2 — fp8 DoubleRow,
split-batch software pipelining.

Per-core: B_local=128 batch rows, H=512, 64 steps, data-parallel over 8
cores (weights replicated, host pre-transposed/pre-quantized).

Layout "T": features on partitions, batch on the free dim. The 128 batch
rows are processed as TWO independent 64-row halves whose recurrences
interleave on the engines: while one half's tail chain (sigmoid -> c update
-> tanh -> h writes) runs on ACT/DVE, the other half's matmul streams run
on PE. This hides the serial per-layer tail latency that otherwise floors
the step time.

Numerics (validated vs reference in numpy sim, 6.2e-3):
  - i/f/o gate matmuls: fp8 e4m3 DoubleRow (K=256/instr, 0.5 cyc/row),
    weights x16; h quantized to fp8 per step.
  - g (tanh) gate matmuls: bf16 x bf16 (the tanh path dominates the error
    budget; sigmoid paths tolerate fp8 noise).
  - y feedback: in(t) = [y(t-1), x(t)], y(t-1) = s + bp with s = Wp h1.
    The y contribution to L0 gates is outer(wcol, s + bp + zb_y), emitted
    as one K=1 DoubleRow matmul per chunk against fp8 (hi, lo) rows of
    16*wcol, with s8 = fp8(s + bp + zb_y) from the proj result. Bias b1
    uses the same (hi, lo)-pair trick against a fp8 ones vector.
  - PSUM uniformly 16x; gate ACTs apply scale=1/16. Tail elementwise fp16
    (DVE 2x); c state fp16; h written as bf16 (g-rhs + proj) and fp8.

PSUM per (layer, half): one [128, 1024] f32 tile (2 banks); columns are
16 chunk-slots of 64 in order i(0-3) f(4-7) o(8-11) g(12-15), so one
768-wide sigmoid covers i|f|o and one ACT covers g. Start/stop are per
2KB bank: bank1 (o+g) closes first (tanh g fires early), bank0 (i+f)
closes last with the sigmoid + c-chain behind it.
"""

import numpy as np
from contextlib import ExitStack

import concourse.bass as bass
import concourse.bacc as bacc
import concourse.mybir as mybir
import concourse.tile as tile

F32 = mybir.dt.float32
F32R = mybir.dt.float32r
BF16 = mybir.dt.bfloat16
F16 = mybir.dt.float16
F8 = mybir.dt.float8e4
AF = mybir.ActivationFunctionType
OP = mybir.AluOpType
DRM = mybir.MatmulPerfMode.DoubleRow

P = 128           # batch rows per core
HB = 64           # batch rows per half
H = 512           # hidden
G = 2048          # 4*H gates
KC = 4            # K chunks of 128 across H
S = 64            # steps
EXO = 8
ZD = 16
SCL = 16.0        # weight scale baked into all gate-stream weights

# pytorch gate order by chunk: i: 0-3, f: 4-7, g: 8-11, o: 12-15
G_CHUNKS = (8, 9, 10, 11)


def slot(c):
    """column slot of chunk c in the [P, 1024] half-tile: i 0-3, f 4-7,
    o 8-11, g 12-15 (so i|f|o are contiguous for the merged sigmoid)."""
    return c if c < 8 else (c - 4 if c >= 12 else c + 4)


CH_B1 = [12, 13, 14, 15, 8, 9, 10, 11]  # bank1: o then g; stop at chunk 11
CH_B0 = [0, 1, 2, 3, 4, 5, 6, 7]        # bank0: i then f; stop at chunk 7
CH_ALL = CH_B1 + CH_B0

ROLES = {}  # instruction name -> role string (diagnostics only)
# ordered per-opcode role lists (diagnostics: trace slices pair up in order)
ORD = {"Matmult": [], "Activation": [], "TensorTensor": [], "TensorCopy": [],
       "Memset": []}


def _reg(kind, role):
    ORD[kind].append(role)


def _tag_role(inst, role):
    try:
        ROLES[inst.ins.name] = role
    except Exception:
        pass
    return inst


def build_kernel(nc: bass.Bass, steps: int):
    assert steps == S
    def di(name, shape, dt):
        return nc.dram_tensor(name, shape, dt, kind="ExternalInput").ap()

    # fp8 i/f/o weights, [p, k*G+g] = 16*W[g, k*128+p]
    w8h0 = di("w8h0", [P, KC * G], F8)
    w8i1 = di("w8i1", [P, KC * G], F8)
    w8h1 = di("w8h1", [P, KC * G], F8)
    # bf16 g-gate weights, [p, k*512+j] = 16*W[1024+j, k*128+p]
    wg0 = di("wg0", [P, KC * H], BF16)
    wgi1 = di("wgi1", [P, KC * H], BF16)
    wgh1 = di("wgh1", [P, KC * H], BF16)
    # L0 input weights: rows 0-7 exo cols x16, row 8 = 16*b0, row 9 = 16*wcol
    wa0 = di("wa0", [10, G], BF16)
    b1p = di("b1p", [1, 2 * G], F8)           # (hi, lo) of 16*b1
    wcp = di("wcp", [1, 2 * G], F8)           # (hi, lo) of 16*wcol
    wpT = di("wpT", [P, KC], F32)             # [p, k] = W_proj[0, k*128+p]
    bp = di("bp", [1, 1], F32)
    wzT = di("wzT", [ZD, 9], F32)             # W_z.T
    bz8 = di("bz8", [8, 1], F32)              # b_z[1:9]
    bz0 = di("bz0", [1, 1], F32)              # b_z[0]
    ones_row = di("ones_row", [1, S * P], BF16)
    zT = di("zT", [ZD, P], F32)               # z.T
    y0T = di("y0T", [1, P], F32)
    xfT = di("xfT", [EXO, S * P], BF16)       # [e, t*128+b] = x_future[b, t, e]
    # per-half states, [p, k*64+b] = h[b_global = k*128 + u*64 + b]
    h0b_d = [di(f"h0b{u}", [P, H // 2], BF16) for u in (0, 1)]
    h1b_d = [di(f"h1b{u}", [P, H // 2], BF16) for u in (0, 1)]
    h08_d = [di(f"h08{u}", [P, H // 2], F8) for u in (0, 1)]
    h18_d = [di(f"h18{u}", [P, H // 2], F8) for u in (0, 1)]
    c0_d = [di(f"c0T{u}", [P, H // 2], F16) for u in (0, 1)]
    c1_d = [di(f"c1T{u}", [P, H // 2], F16) for u in (0, 1)]
    out_d = nc.dram_tensor("out", [S, P], F32, kind="ExternalOutput").ap()

    with tile.TileContext(nc) as tc, ExitStack() as ctx:
        emit(ctx, tc, nc, locals())
    return nc


def emit(ctx, tc, nc, t_in):
    w8h0, w8i1, w8h1 = t_in["w8h0"], t_in["w8i1"], t_in["w8h1"]
    wg0, wgi1, wgh1 = t_in["wg0"], t_in["wgi1"], t_in["wgh1"]
    wa0, b1p, wcp, wpT, bp = (t_in["wa0"], t_in["b1p"], t_in["wcp"],
                              t_in["wpT"], t_in["bp"])
    wzT, bz8, bz0, zT = t_in["wzT"], t_in["bz8"], t_in["bz0"], t_in["zT"]
    ones_row, y0T, xfT = t_in["ones_row"], t_in["y0T"], t_in["xfT"]
    h0b_d, h1b_d, h08_d, h18_d = (t_in["h0b_d"], t_in["h1b_d"],
                                  t_in["h08_d"], t_in["h18_d"])
    c0_d, c1_d, out_d = t_in["c0_d"], t_in["c1_d"], t_in["out_d"]

    # ---- pools ----
    const = ctx.enter_context(tc.tile_pool(name="const", bufs=1))
    ldtmp = ctx.enter_context(tc.tile_pool(name="ldtmp", bufs=1))
    state = ctx.enter_context(tc.tile_pool(name="state", bufs=2))
    act = ctx.enter_context(tc.tile_pool(name="act", bufs=2))
    yo = ctx.enter_context(tc.tile_pool(name="yo", bufs=3))
    g0p = ctx.enter_context(tc.tile_pool(name="g0p", bufs=1, space="PSUM"))
    g1p = ctx.enter_context(tc.tile_pool(name="g1p", bufs=1, space="PSUM"))

    dma = nc.sync.dma_start
    adma = nc.scalar.dma_start
    gdma = nc.gpsimd.dma_start

    # ---- persistent SBUF ----
    W8h0 = const.tile([P, KC * G], F8, name="W8h0")
    W8i1 = const.tile([P, KC * G], F8, name="W8i1")
    W8h1 = const.tile([P, KC * G], F8, name="W8h1")
    Wg0 = const.tile([P, KC * H], BF16, name="Wg0")
    Wgi1 = const.tile([P, KC * H], BF16, name="Wgi1")
    Wgh1 = const.tile([P, KC * H], BF16, name="Wgh1")
    wa0_sb = const.tile([10, G], BF16, name="wa0_sb")
    b1p_sb = const.tile([1, 2 * G], F8, name="b1p_sb")
    wcp_sb = const.tile([1, 2 * G], F8, name="wcp_sb")
    ones8 = const.tile([1, 2 * HB], F8, name="ones8")
    wpT_sb = const.tile([P, KC], BF16, name="wpT_sb")
    inT = const.tile([10, S * P], BF16, name="inT")  # p0-7 exo, p8 ones, p9 y0
    zb8 = const.tile([8, P], BF16, name="zb8")     # z-bias for exo rows
    zby = const.tile([1, P], F32, name="zby")      # z-bias for the y slot
    bz8_sb = const.tile([8, 1], F32, name="bz8_sb")
    bz0_sb = const.tile([1, 1], F32, name="bz0_sb")
    bp_row = const.tile([1, P], F32, name="bp_row")
    zbybp = const.tile([1, P], F32, name="zbybp")  # zb_y + bp

    # 3D views used by DR matmuls
    W8h0v = W8h0.rearrange("p (k g) -> p k g", k=KC)
    W8i1v = W8i1.rearrange("p (k g) -> p k g", k=KC)
    W8h1v = W8h1.rearrange("p (k g) -> p k g", k=KC)
    b1v = b1p_sb.rearrange("o (two g) -> o two g", two=2)
    wcv = wcp_sb.rearrange("o (two g) -> o two g", two=2)
    on8v = ones8.rearrange("o (two b) -> o two b", two=2)

    # ---- init loads ----
    # SP queue: Wg0+W8h0 first (gate the step-0 A pass), then wa0
    dma(Wg0[:], wg0)
    dma(W8h0[:], w8h0)
    dma(wa0_sb[:], wa0)
    # gpsimd queue: z tensors (f32r cast loads), states, xfT, then L1 weights
    wzT_sb = ldtmp.tile([ZD, 9], F32R, name="wzT_sb", tag="wz")
    gdma(wzT_sb[:], wzT)
    zT_sb = ldtmp.tile([ZD, P], F32R, name="zT_sb", tag="zt")
    gdma(zT_sb[:], zT)
    def half_states(pfx, dt):
        return [state.tile([P, H // 2], dt, name=f"{pfx}{u}", tag=f"{pfx}{u}")
                for u in (0, 1)]

    h0b_c = half_states("h0b", BF16)
    h08_c = half_states("h08", F8)
    h1b_c = half_states("h1b", BF16)
    h18_c = half_states("h18", F8)
    c0_c = half_states("c0", F16)
    c1_c = half_states("c1", F16)
    for u in (0, 1):
        gdma(h0b_c[u][:], h0b_d[u])
        gdma(h08_c[u][:], h08_d[u])
    xfT_sb = ldtmp.tile([EXO, S * P], BF16, name="xfT_sb", tag="xf")
    gdma(xfT_sb[:], xfT)
    for u in (0, 1):
        gdma(c0_c[u][:], c0_d[u])
        gdma(h1b_c[u][:], h1b_d[u])
        gdma(h18_c[u][:], h18_d[u])
        gdma(c1_c[u][:], c1_d[u])
    gdma(Wgh1[:], wgh1)
    gdma(W8h1[:], w8h1)
    gdma(Wgi1[:], wgi1)
    gdma(W8i1[:], w8i1)
    # ACT queue: dummy sigmoid first so the table set loads immediately,
    # then the small admas that gate the zb ACTs
    dumm = ldtmp.tile([1, 1], F32, name="dumm", tag="dumm")
    _reg("Memset", "dumm")
    nc.vector.memset(dumm[:], 0.0)
    _reg("Activation", "dumm")
    nc.scalar.activation(dumm[:], dumm[:], AF.Sigmoid)
    adma(bz8_sb[:], bz8)
    adma(bz0_sb[:], bz0)
    y0T_sb = ldtmp.tile([1, P], F32, name="y0T_sb", tag="y0")
    adma(y0T_sb[:], y0T)
    adma(inT[8:9, :], ones_row)  # b0 ones row (host const)
    adma(b1p_sb[:], b1p)
    adma(wcp_sb[:], wcp)

    _reg("Memset", "ones8")
    nc.vector.memset(ones8[:], 1.0)

    # ---- z bias: zb8[8, P] = W_z[1:9] @ z.T + b_z[1:9]; zby = row 0 ----
    zb_ps = g0p.tile([P, 2 * H], F32, name="zbps", tag="u0")
    _reg("Matmult", "zb")
    nc.tensor.matmul(
        zb_ps[0:8, 0:P], wzT_sb[:, 1:9], zT_sb[:],
        start=True, stop=True,
    )
    _reg("Matmult", "zb")
    nc.tensor.matmul(
        zb_ps[0:1, P : 2 * P], wzT_sb[:, 0:1], zT_sb[:],
        start=True, stop=True,
    )
    _reg("Activation", "zb8")
    nc.scalar.activation(zb8[:], zb_ps[0:8, 0:P], AF.Identity, bias=bz8_sb[:])
    _reg("Activation", "zby")
    nc.scalar.activation(zby[:], zb_ps[0:1, P : 2 * P], AF.Identity, bias=bz0_sb[:])

    # rest of the ACT-queue loads (after the zb ACTs so they don't gate them)
    bp_sb = ldtmp.tile([1, 1], F32, name="bp_sb", tag="bp")
    adma(bp_sb[:], bp)
    wpf = ldtmp.tile([P, KC], F32, name="wpf", tag="wp")
    adma(wpf[:], wpT)
    _reg("TensorCopy", "wpT")
    nc.vector.tensor_copy(wpT_sb[:], wpf[:])  # f32 -> bf16

    # exo rows with z-bias baked in: write straight into inT rows 0-7
    SPLIT = 4
    x3 = xfT_sb.rearrange("e (t b) -> e t b", b=P)
    dst3 = inT[0:8, :].rearrange("e (t b) -> e t b", b=P)
    zb3a = zb8.unsqueeze(1).broadcast_to((EXO, SPLIT, P))
    zb3b = zb8.unsqueeze(1).broadcast_to((EXO, S - SPLIT, P))
    _reg("TensorTensor", "exo_a")
    nc.vector.tensor_tensor(dst3[:, 0:SPLIT], x3[:, 0:SPLIT], zb3a, op=OP.add)
    # y0 row (partition 9, engine-unwritable): staged base-0 then DMA'd.
    # Only block 0 is ever read (t=0 uses K=10; t>0 uses K=9).
    fr0 = ldtmp.tile([1, P], BF16, name="fr0", tag="fr0")
    _reg("TensorTensor", "fr0")
    nc.vector.tensor_tensor(fr0[:], y0T_sb[:], zby[:], op=OP.add)
    adma(inT[9:10, 0:P], fr0[:])
    _reg("TensorCopy", "bp_row")
    nc.vector.tensor_copy(bp_row[:], bp_sb[0:1, 0:1].broadcast_to((1, P)))
    _reg("TensorTensor", "zbybp")
    nc.vector.tensor_tensor(zbybp[:], zby[:], bp_row[:], op=OP.add)
    _reg("TensorTensor", "exo_b")
    nc.vector.tensor_tensor(dst3[:, SPLIT:S], x3[:, SPLIT:S], zb3b, op=OP.add)

    # =========================== STEP LOOP ===========================
    CUR = ["init"]

    def R(role):
        CUR[0] = role

    def mm(*a, **k):
        _reg("Matmult", CUR[0])
        return _tag_role(nc.tensor.matmul(*a, **k), CUR[0])

    def sact(*a, **k):
        _reg("Activation", CUR[0])
        return _tag_role(nc.scalar.activation(*a, **k), CUR[0])

    def vtt(*a, **k):
        _reg("TensorTensor", CUR[0])
        return _tag_role(nc.vector.tensor_tensor(*a, **k), CUR[0])

    def g_tile(pool, u, pfx):
        return pool.tile([P, 2 * H], F32, name=pfx + f"u{u}", tag=f"u{u}")

    def hv(t4):
        """[P, KC*HB]-layout half state -> [P, KC, HB] view."""
        return t4.rearrange("p (k b) -> p k b", b=HB)

    def bkey(c):
        return "b0" if slot(c) < 8 else "b1"

    def dst_of(gt, c):
        s = slot(c)
        return gt[:, s * HB : (s + 1) * HB]

    def hid_mms(gt, c, W8v, Wg, h8u, hbu, started, stop=False):
        """Hidden-side matmuls for one gate chunk of one half: fp8 DR for
        i/f/o, bf16 for g. One PSUM start per bank (zero region)."""
        dst = dst_of(gt, c)
        bk = bkey(c)
        if c in G_CHUNKS:
            j0 = (c - 8) * P
            hb3 = hv(hbu)
            for k in range(KC):
                mm(
                    dst, Wg[:, k * H + j0 : k * H + j0 + P],
                    hb3[:, k : k + 1, :],
                    start=not started.get(bk, False),
                    stop=(stop and k == KC - 1),
                )
                started[bk] = True
        else:
            h83 = hv(h8u)
            for k2 in range(2):
                mm(
                    dst, W8v[:, 2 * k2 : 2 * k2 + 2, c * P : (c + 1) * P],
                    h83[:, 2 * k2 : 2 * k2 + 2, :],
                    start=not started.get(bk, False),
                    stop=(stop and k2 == 1),
                    perf_mode=DRM,
                )
                started[bk] = True

    def emit_A(gt, h08u, h0bu, started):
        R("A")
        for c in CH_ALL:
            hid_mms(gt, c, W8h0v, Wg0, h08u, h0bu, started)

    def emit_B(gt, t, u, parts=None):
        """L0 input matmuls: K=10 at t=0 (y0 row), K=9 after (exo + ones).
        With parts (t=0 only): carries the bank stops."""
        K = 10 if t == 0 else 9
        rhs = inT[0:K, t * P + u * HB : t * P + u * HB + HB]
        if parts is None:
            R("B")
            for c in CH_ALL:
                mm(dst_of(gt, c), wa0_sb[0:K, c * P : (c + 1) * P], rhs,
                   start=False, stop=False)
        else:
            R("B")
            for c in CH_B1:
                mm(dst_of(gt, c), wa0_sb[0:K, c * P : (c + 1) * P], rhs,
                   start=False, stop=(c == 11))
            parts[0]()
            R("B")
            for c in CH_B0:
                mm(dst_of(gt, c), wa0_sb[0:K, c * P : (c + 1) * P], rhs,
                   start=False, stop=(c == 7))
            parts[1]()  # sg only; caller runs the rest via run_tails

    def emit_feed(gt, s8v, parts):
        """y-feedback: one K=1 DR outer product per chunk against the fp8
        (hi, lo) wcol rows; carries the bank stops; tail parts interleave."""
        R("feed")
        for c in CH_B1:
            mm(dst_of(gt, c), wcv[:, :, c * P : (c + 1) * P], s8v,
               start=False, stop=(c == 11), perf_mode=DRM)
        parts[0]()
        R("feed")
        for c in CH_B0:
            mm(dst_of(gt, c), wcv[:, :, c * P : (c + 1) * P], s8v,
               start=False, stop=(c == 7), perf_mode=DRM)
        parts[1]()

    def emit_C(gt, h18u, h1bu, started):
        """L1 h1-part matmuls + fp8 DR bias (start-side)."""
        for c in CH_ALL:
            R("C")
            hid_mms(gt, c, W8h1v, Wgh1, h18u, h1bu, started)
            R("Cb")
            mm(dst_of(gt, c), b1v[:, :, c * P : (c + 1) * P],
               on8v[:], start=False, stop=False, perf_mode=DRM)

    def emit_D(gt, h08u, h0bu, parts):
        """L1 h0-part matmuls: the bank-closing stops; tail parts
        interleave between bank groups."""
        R("D")
        for c in CH_B1:
            hid_mms(gt, c, W8i1v, Wgi1, h08u, h0bu, {bkey(c): True},
                    stop=(c == 11))
        parts[0]()
        R("D")
        for c in CH_B0:
            hid_mms(gt, c, W8i1v, Wgi1, h08u, h0bu, {bkey(c): True},
                    stop=(c == 7))
        parts[1]()

    def make_tail(gt, c_prev_u, u, c_n_u, h_b_u, h_8_u, htag, h8_first):
        """Tail for one (layer, half) as 5 staged callbacks (tg, sg, c-chain,
        tanh(c), h-writes). The caller interleaves stages across the two
        halves so neither half's pending op blocks the other's ready op on
        the in-order ACT/DVE queues."""
        sg = act.tile([P, 3 * H // 2], F16, name=f"sg{u}", tag=f"sg{u}")
        tgt = act.tile([P, H // 2], F16, name=f"tg{u}", tag=f"tg{u}")
        t1 = act.tile([P, H // 2], F16, name=f"t1{u}", tag=f"t1{u}")
        t2 = act.tile([P, H // 2], F16, name=f"t2{u}", tag=f"t2{u}")
        thc = act.tile([P, H // 2], F16, name=f"thc{u}", tag=f"thc{u}")
        cpv = hv(c_prev_u)
        cnv = hv(c_n_u)
        hbv = hv(h_b_u)
        h8v = hv(h_8_u)
        sg3 = sg.rearrange("p (s b) -> p s b", b=HB)
        sgi, sgf, sgo = sg3[:, 0:4], sg3[:, 4:8], sg3[:, 8:12]
        tg3 = tgt.rearrange("p (s b) -> p s b", b=HB)
        t13 = t1.rearrange("p (s b) -> p s b", b=HB)
        t23 = t2.rearrange("p (s b) -> p s b", b=HB)
        th3 = thc.rearrange("p (s b) -> p s b", b=HB)
        ISC = 1.0 / SCL

        def p_g():
            R(htag + f":tg{u}")
            sact(tg3[:], gt[:, 12 * HB : 16 * HB].rearrange(
                "p (s b) -> p s b", b=HB), AF.Tanh, scale=ISC)

        def p_sg():
            R(htag + f":sg{u}")
            sact(sg[:], gt[:, 0 : 12 * HB], AF.Sigmoid, scale=ISC)

        def p_chain():
            R(htag + f":cchain{u}")
            vtt(t23[:], sgf, cpv, op=OP.mult)
            vtt(t13[:], sgi, tg3[:], op=OP.mult)
            vtt(cnv, t13[:], t23[:], op=OP.add)

        def p_thc():
            R(htag + f":thc{u}")
            sact(th3[:], cnv, AF.Tanh)

        def p_hwr():
            R(htag + f":hwr{u}")
            outs = [h8v, hbv] if h8_first else [hbv, h8v]
            for dst in outs:
                vtt(dst, sgo, th3[:], op=OP.mult)

        return [p_g, p_sg, p_chain, p_thc, p_hwr]

    def run_tails(tails):
        """Interleave the two halves' later tail stages: ACT gets
        sg0, sg1, thc0, thc1 while DVE runs chain0, chain1, hwr0, hwr1."""
        tails[0][2](); tails[1][2]()
        tails[0][3](); tails[1][3]()
        tails[0][4](); tails[1][4]()

    # prologue: step-0 L0 gates for both halves (+tails via B stops), L1 start
    g0_cur = [g_tile(g0p, u, "g0") for u in (0, 1)]
    for u in (0, 1):
        emit_A(g0_cur[u], h08_c[u], h0b_c[u], {})
    c0_n = half_states("c0", F16)
    h0b_n = half_states("h0b", BF16)
    h08_n = half_states("h08", F8)
    tails = [make_tail(g0_cur[u], c0_c[u], u, c0_n[u], h0b_n[u], h08_n[u],
                       "h0", h8_first=True) for u in (0, 1)]
    for u in (0, 1):
        emit_B(g0_cur[u], 0, u, tails[u])
    run_tails(tails)
    c0_c, h0b_cur, h08_cur = c0_n, h0b_n, h08_n
    g1_cur = [g_tile(g1p, u, "g1") for u in (0, 1)]
    for u in (0, 1):
        emit_C(g1_cur[u], h18_c[u], h1b_c[u], {})

    for t in range(S):
        # ---- L1 gates close for t (D), tails interleaved, both halves ----
        c1_n = half_states("c1", F16)
        h1b_n = half_states("h1b", BF16)
        h18_n = half_states("h18", F8)
        tails1 = [make_tail(g1_cur[u], c1_c[u], u, c1_n[u], h1b_n[u],
                            h18_n[u], "h1", h8_first=False) for u in (0, 1)]
        for u in (0, 1):
            R("D")
            for c in CH_B1:
                hid_mms(g1_cur[u], c, W8i1v, Wgi1, h08_cur[u], h0b_cur[u],
                        {bkey(c): True}, stop=(c == 11))
            tails1[u][0]()
        for u in (0, 1):
            R("D")
            for c in CH_B0:
                hid_mms(g1_cur[u], c, W8i1v, Wgi1, h08_cur[u], h0b_cur[u],
                        {bkey(c): True}, stop=(c == 7))
            tails1[u][1]()
        run_tails(tails1)
        c1_c = c1_n
        # ---- L0 gates for t+1: hidden + input (start-side) ----
        if t + 1 < S:
            g0_nxt = [g_tile(g0p, u, "g0") for u in (0, 1)]
            for u in (0, 1):
                st = {}
                emit_A(g0_nxt[u], h08_cur[u], h0b_cur[u], st)
                emit_B(g0_nxt[u], t + 1, u)
        # ---- proj + y outputs + s8 per half ----
        s8vs = []
        for u in (0, 1):
            u0, u1 = u * HB, u * HB + HB
            h1b3 = hv(h1b_n[u])
            pj = g1_cur[u][0:1, 11 * HB : 12 * HB]
            R("proj")
            for k in range(KC):
                mm(pj, wpT_sb[:, k : k + 1], h1b3[:, k : k + 1, :],
                   start=(k == 0), stop=(k == KC - 1))
            s8 = yo.tile([1, 2 * HB], F8, name=f"s8{u}", tag=f"s8{u}")
            s8v = s8.rearrange("o (two b) -> o two b", two=2)
            R("s8")
            vtt(s8v[:],
                pj.unsqueeze(1).broadcast_to((1, 2, HB)),
                zbybp[:, u0:u1].unsqueeze(1).broadcast_to((1, 2, HB)),
                op=OP.add)
            s8vs.append(s8v)
            yo_t = yo.tile([1, HB], F32, name=f"yout{u}", tag=f"yout{u}")
            R("y_out")
            vtt(yo_t[:], pj, bp_row[:, u0:u1], op=OP.add)
            dma(out_d[t : t + 1, u0:u1], yo_t[:])
        # ---- L0 closes for t+1 (feed), tails interleaved; L1 starts ----
        if t + 1 < S:
            c0_n = half_states("c0", F16)
            h0b_n = half_states("h0b", BF16)
            h08_n = half_states("h08", F8)
            tails0 = [make_tail(g0_nxt[u], c0_c[u], u, c0_n[u], h0b_n[u],
                                h08_n[u], "h0", h8_first=True) for u in (0, 1)]
            for u in (0, 1):
                R("feed")
                for c in CH_B1:
                    mm(dst_of(g0_nxt[u], c), wcv[:, :, c * P : (c + 1) * P],
                       s8vs[u], start=False, stop=(c == 11), perf_mode=DRM)
                tails0[u][0]()
            for u in (0, 1):
                R("feed")
                for c in CH_B0:
                    mm(dst_of(g0_nxt[u], c), wcv[:, :, c * P : (c + 1) * P],
                       s8vs[u], start=False, stop=(c == 7), perf_mode=DRM)
                tails0[u][1]()
            run_tails(tails0)
            c0_c, h0b_cur, h08_cur = c0_n, h0b_n, h08_n
            g1_nxt = [g_tile(g1p, u, "g1") for u in (0, 1)]
            for u in (0, 1):
                emit_C(g1_nxt[u], h18_n[u], h1b_n[u], {})
            g0_cur, g1_cur = g0_nxt, g1_nxt


def make_nc(steps: int = S):
    nc = bacc.Bacc("TRN2", target_bir_lowering=False, debug=False)
    build_kernel(nc, steps)
    nc.compile()
    return nc


# ======================= host-side prep =======================

def _bf16(x):
    import ml_dtypes
    return np.ascontiguousarray(np.asarray(x, np.float32).astype(ml_dtypes.bfloat16))


def _f8(x):
    import ml_dtypes
    return np.ascontiguousarray(np.asarray(x, np.float32).astype(ml_dtypes.float8_e4m3fn))


def _f8pair(v):
    """f32 vector -> [1, 2*len] fp8 (hi row, lo row) of 16*v."""
    import ml_dtypes
    v = np.asarray(v, np.float32) * SCL
    hi = v.astype(ml_dtypes.float8_e4m3fn)
    lo = (v - hi.astype(np.float32)).astype(ml_dtypes.float8_e4m3fn)
    return np.ascontiguousarray(np.stack([hi, lo]).reshape(1, 2 * v.size))


def _f32(x):
    return np.ascontiguousarray(np.asarray(x, dtype=np.float32))


def _wT(W):
    """[G', H] f32 -> [128, KC*G'] with [p, k*G'+g] = W[g, k*128+p]."""
    W = np.asarray(W, dtype=np.float32)
    Gp = W.shape[0]
    return W.T.reshape(KC, P, Gp).transpose(1, 0, 2).reshape(P, KC * Gp)


def _hT4(h):
    """[B_loc, H] -> [128, H] with [p, k*128+b] = h[b, k*128+p]."""
    return h.T.reshape(KC, P, P).transpose(1, 0, 2).reshape(P, H)


def shard_inputs(inputs, steps: int):
    B = inputs["y0"].shape[0]
    nb = B // P
    Wih0 = np.asarray(inputs["W_ih0"], np.float32)
    Wp = np.asarray(inputs["W_proj"], np.float32)
    b0 = _f32(inputs["b_ih0"]) + _f32(inputs["b_hh0"])
    b1 = _f32(inputs["b_ih1"]) + _f32(inputs["b_hh1"])
    wa0 = np.concatenate(
        [
            Wih0[:, 1:9].T * SCL,          # exo rows
            b0.reshape(1, G) * SCL,        # ones-row bias
            Wih0[:, 0:1].T * SCL,          # y0 row (t=0 only)
        ],
        axis=0,
    )

    def wsplit(W):
        W = np.asarray(W, np.float32) * SCL
        return _f8(_wT(W)), _bf16(_wT(W[1024:1536]))

    w8h0, wg0 = wsplit(inputs["W_hh0"])
    w8i1, wgi1 = wsplit(inputs["W_ih1"])
    w8h1, wgh1 = wsplit(inputs["W_hh1"])
    rep = {
        "w8h0": w8h0, "wg0": wg0,
        "w8i1": w8i1, "wgi1": wgi1,
        "w8h1": w8h1, "wgh1": wgh1,
        "wa0": _bf16(wa0),
        "b1p": _f8pair(b1),
        "wcp": _f8pair(Wih0[:, 0]),
        "wpT": _f32(Wp).reshape(KC, P).T.copy(),
        "bp": _f32(inputs["b_proj"]).reshape(1, 1),
        "wzT": _f32(inputs["W_z"]).T.copy(),
        "bz8": _f32(inputs["b_z"]).reshape(9, 1)[1:9].copy(),
        "bz0": _f32(inputs["b_z"]).reshape(9, 1)[0:1].copy(),
        "ones_row": _bf16(np.ones((1, S * P), np.float32)),
    }
    maps = []
    for i in range(nb):
        s = slice(i * P, (i + 1) * P)
        x = np.asarray(inputs["x_future"], np.float32)[s, :steps]  # [P, S, E]
        h0b = _bf16(_hT4(np.asarray(inputs["h0"], np.float32)[0, s]))
        h1b = _bf16(_hT4(np.asarray(inputs["h0"], np.float32)[1, s]))
        c0h = _hT4(np.asarray(inputs["c0"], np.float32)[0, s]).astype(np.float16)
        c1h = _hT4(np.asarray(inputs["c0"], np.float32)[1, s]).astype(np.float16)

        def hsplit(full, u):
            """[P, 512] = [p, k*128+b] -> half [P, 256] = [p, k*64+b]"""
            v = np.asarray(full).reshape(P, KC, P)[:, :, u * HB : (u + 1) * HB]
            return np.ascontiguousarray(v.reshape(P, KC * HB))

        m = dict(rep)
        m.update(
            {
                "zT": _f32(inputs["z"][s]).T.copy(),
                "y0T": _f32(inputs["y0"][s]).reshape(1, P).copy(),
                "xfT": _bf16(x.transpose(2, 1, 0).reshape(EXO, steps * P)),
            }
        )
        for u in (0, 1):
            m[f"h0b{u}"] = hsplit(h0b, u)
            m[f"h1b{u}"] = hsplit(h1b, u)
            m[f"h08{u}"] = hsplit(_f8(h0b.astype(np.float32)), u)
            m[f"h18{u}"] = hsplit(_f8(h1b.astype(np.float32)), u)
            m[f"c0T{u}"] = hsplit(c0h, u)
            m[f"c1T{u}"] = hsplit(c1h, u)
        maps.append(m)
    return maps


def assemble_output(results, steps: int):
    outs = [
        np.ascontiguousarray(np.asarray(rm["out"]).T).reshape(P, steps, 1)
        for rm in results
    ]
    return np.concatenate(outs, axis=0)


# ======================= public entry point =======================
_NC_CACHE = {}


def _get_nc():
    if "nc" not in _NC_CACHE:
        _NC_CACHE["nc"] = make_nc(S)
    return _NC_CACHE["nc"]


STEPS = S
N_CORES = 8


def kernel(**inputs):
    """Full-input entry point: shards batch over 8 NeuronCores, runs the
    Bass LSTM-decoder kernel, reassembles [B, steps, 1] float32 output."""
    from concourse.bass_utils import run_bass_kernel_spmd

    steps = int(inputs.get("steps", STEPS))
    assert steps == STEPS, f"kernel compiled for {STEPS} steps, got {steps}"
    nc = _get_nc()
    maps = shard_inputs(inputs, STEPS)
    res = run_bass_kernel_spmd(nc, maps, list(range(N_CORES)))
    return assemble_output(res.results, STEPS).astype(np.float32)


# revision 15
# speedup vs baseline: 1.1675x; 1.1066x over previous
"""2-layer LSTM decoder Bass/Tile kernel for TRN2 — fp8 DoubleRow,
split-batch software pipelining.

Per-core: B_local=128 batch rows, H=512, 64 steps, data-parallel over 8
cores (weights replicated, host pre-transposed/pre-quantized).

Layout "T": features on partitions, batch on the free dim. The 128 batch
rows are processed as TWO independent 64-row halves whose recurrences
interleave on the engines: while one half's tail chain (sigmoid -> c update
-> tanh -> h writes) runs on ACT/DVE, the other half's matmul streams run
on PE. This hides the serial per-layer tail latency that otherwise floors
the step time.

Numerics (validated vs reference in numpy sim, 6.2e-3):
  - i/f/o gate matmuls: fp8 e4m3 DoubleRow (K=256/instr, 0.5 cyc/row),
    weights x16; h quantized to fp8 per step.
  - g (tanh) gate matmuls: bf16 x bf16 (the tanh path dominates the error
    budget; sigmoid paths tolerate fp8 noise).
  - y feedback: in(t) = [y(t-1), x(t)], y(t-1) = s + bp with s = Wp h1.
    The y contribution to L0 gates is outer(wcol, s + bp + zb_y), emitted
    as one K=1 DoubleRow matmul per chunk against fp8 (hi, lo) rows of
    16*wcol, with s8 = fp8(s + bp + zb_y) from the proj result. Bias b1
    uses the same (hi, lo)-pair trick against a fp8 ones vector.
  - PSUM uniformly 16x; gate ACTs apply scale=1/16. Tail elementwise fp16
    (DVE 2x); c state fp16; h written as bf16 (g-rhs + proj) and fp8.

PSUM per (layer, half): one [128, 1024] f32 tile (2 banks); columns are
16 chunk-slots of 64 in order i(0-3) f(4-7) o(8-11) g(12-15), so one
768-wide sigmoid covers i|f|o and one ACT covers g. Start/stop are per
2KB bank: bank1 (o+g) closes first (tanh g fires early), bank0 (i+f)
closes last with the sigmoid + c-chain behind it.
"""

import numpy as np
from contextlib import ExitStack

import concourse.bass as bass
import concourse.bacc as bacc
import concourse.mybir as mybir
import concourse.tile as tile

F32 = mybir.dt.float32
F32R = mybir.dt.float32r
BF16 = mybir.dt.bfloat16
F16 = mybir.dt.float16
F8 = mybir.dt.float8e4
AF = mybir.ActivationFunctionType
OP = mybir.AluOpType
DRM = mybir.MatmulPerfMode.DoubleRow

P = 128           # batch rows per core
HB = 64           # batch rows per half
H = 512           # hidden
G = 2048          # 4*H gates
KC = 4            # K chunks of 128 across H
S = 64            # steps
EXO = 8
ZD = 16
SCL = 16.0        # weight scale baked into all gate-stream weights

# pytorch gate order by chunk: i: 0-3, f: 4-7, g: 8-11, o: 12-15
G_CHUNKS = (8, 9, 10, 11)


def slot(c):
    """column slot of chunk c in the [P, 1024] half-tile: i 0-3, f 4-7,
    o 8-11, g 12-15 (so i|f|o are contiguous for the merged sigmoid)."""
    return c if c < 8 else (c - 4 if c >= 12 else c + 4)


CH_B1 = [12, 13, 14, 15, 8, 9, 10, 11]  # bank1: o then g; stop at chunk 11
CH_B0 = [0, 1, 2, 3, 4, 5, 6, 7]        # bank0: i then f; stop at chunk 7
CH_ALL = CH_B1 + CH_B0

ROLES = {}  # instruction name -> role string (diagnostics only)
# ordered per-opcode role lists (diagnostics: trace slices pair up in order)
ORD = {"Matmult": [], "Activation": [], "TensorTensor": [], "TensorCopy": [],
       "Memset": []}


def _reg(kind, role):
    ORD[kind].append(role)


def _tag_role(inst, role):
    try:
        ROLES[inst.ins.name] = role
    except Exception:
        pass
    return inst


def build_kernel(nc: bass.Bass, steps: int):
    assert steps == S
    def di(name, shape, dt):
        return nc.dram_tensor(name, shape, dt, kind="ExternalInput").ap()

    # fp8 i/f/o weights, [p, k*G+g] = 16*W[g, k*128+p]
    w8h0 = di("w8h0", [P, KC * G], F8)
    w8i1 = di("w8i1", [P, KC * G], F8)
    w8h1 = di("w8h1", [P, KC * G], F8)
    # bf16 g-gate weights, [p, k*512+j] = 16*W[1024+j, k*128+p]
    wg0 = di("wg0", [P, KC * H], BF16)
    wgi1 = di("wgi1", [P, KC * H], BF16)
    wgh1 = di("wgh1", [P, KC * H], BF16)
    # L0 input weights: rows 0-7 exo cols x16, row 8 = 16*b0, row 9 = 16*wcol
    wa0 = di("wa0", [10, G], BF16)
    b1p = di("b1p", [1, 2 * G], F8)           # (hi, lo) of 16*b1
    wcp = di("wcp", [1, 2 * G], F8)           # (hi, lo) of 16*wcol
    wpT = di("wpT", [P, KC], F32)             # [p, k] = W_proj[0, k*128+p]
    bp = di("bp", [1, 1], F32)
    wzT = di("wzT", [ZD, 9], F32)             # W_z.T
    bz8 = di("bz8", [8, 1], F32)              # b_z[1:9]
    bz0 = di("bz0", [1, 1], F32)              # b_z[0]
    ones_row = di("ones_row", [1, S * P], BF16)
    zT = di("zT", [ZD, P], F32)               # z.T
    y0T = di("y0T", [1, P], F32)
    xfT = di("xfT", [EXO, S * P], BF16)       # [e, t*128+b] = x_future[b, t, e]
    # per-half states, [p, k*64+b] = h[b_global = k*128 + u*64 + b]
    h0b_d = [di(f"h0b{u}", [P, H // 2], BF16) for u in (0, 1)]
    h1b_d = [di(f"h1b{u}", [P, H // 2], BF16) for u in (0, 1)]
    h08_d = [di(f"h08{u}", [P, H // 2], F8) for u in (0, 1)]
    h18_d = [di(f"h18{u}", [P, H // 2], F8) for u in (0, 1)]
    c0_d = [di(f"c0T{u}", [P, H // 2], F16) for u in (0, 1)]
    c1_d = [di(f"c1T{u}", [P, H // 2], F16) for u in (0, 1)]
    out_d = nc.dram_tensor("out", [S, P], F32, kind="ExternalOutput").ap()

    with tile.TileContext(nc) as tc, ExitStack() as ctx:
        emit(ctx, tc, nc, locals())
    return nc


def emit(ctx, tc, nc, t_in):
    w8h0, w8i1, w8h1 = t_in["w8h0"], t_in["w8i1"], t_in["w8h1"]
    wg0, wgi1, wgh1 = t_in["wg0"], t_in["wgi1"], t_in["wgh1"]
    wa0, b1p, wcp, wpT, bp = (t_in["wa0"], t_in["b1p"], t_in["wcp"],
                              t_in["wpT"], t_in["bp"])
    wzT, bz8, bz0, zT = t_in["wzT"], t_in["bz8"], t_in["bz0"], t_in["zT"]
    ones_row, y0T, xfT = t_in["ones_row"], t_in["y0T"], t_in["xfT"]
    h0b_d, h1b_d, h08_d, h18_d = (t_in["h0b_d"], t_in["h1b_d"],
                                  t_in["h08_d"], t_in["h18_d"])
    c0_d, c1_d, out_d = t_in["c0_d"], t_in["c1_d"], t_in["out_d"]

    # ---- pools ----
    const = ctx.enter_context(tc.tile_pool(name="const", bufs=1))
    ldtmp = ctx.enter_context(tc.tile_pool(name="ldtmp", bufs=1))
    state = ctx.enter_context(tc.tile_pool(name="state", bufs=2))
    act = ctx.enter_context(tc.tile_pool(name="act", bufs=2))
    yo = ctx.enter_context(tc.tile_pool(name="yo", bufs=3))
    g0p = ctx.enter_context(tc.tile_pool(name="g0p", bufs=1, space="PSUM"))
    g1p = ctx.enter_context(tc.tile_pool(name="g1p", bufs=1, space="PSUM"))

    dma = nc.sync.dma_start
    adma = nc.scalar.dma_start
    gdma = nc.gpsimd.dma_start

    # ---- persistent SBUF ----
    W8h0 = const.tile([P, KC * G], F8, name="W8h0")
    W8i1 = const.tile([P, KC * G], F8, name="W8i1")
    W8h1 = const.tile([P, KC * G], F8, name="W8h1")
    Wg0 = const.tile([P, KC * H], BF16, name="Wg0")
    Wgi1 = const.tile([P, KC * H], BF16, name="Wgi1")
    Wgh1 = const.tile([P, KC * H], BF16, name="Wgh1")
    wa0_sb = const.tile([10, G], BF16, name="wa0_sb")
    b1p_sb = const.tile([1, 2 * G], F8, name="b1p_sb")
    wcp_sb = const.tile([1, 2 * G], F8, name="wcp_sb")
    ones8 = const.tile([1, 2 * HB], F8, name="ones8")
    wpT_sb = const.tile([P, KC], BF16, name="wpT_sb")
    inT = const.tile([10, S * P], BF16, name="inT")  # p0-7 exo, p8 ones, p9 y0
    zb8 = const.tile([8, P], BF16, name="zb8")     # z-bias for exo rows
    zby = const.tile([1, P], F32, name="zby")      # z-bias for the y slot
    bz8_sb = const.tile([8, 1], F32, name="bz8_sb")
    bz0_sb = const.tile([1, 1], F32, name="bz0_sb")
    bp_row = const.tile([1, P], F32, name="bp_row")
    zbybp = const.tile([1, P], F32, name="zbybp")  # zb_y + bp

    # 3D views used by DR matmuls
    W8h0v = W8h0.rearrange("p (k g) -> p k g", k=KC)
    W8i1v = W8i1.rearrange("p (k g) -> p k g", k=KC)
    W8h1v = W8h1.rearrange("p (k g) -> p k g", k=KC)
    b1v = b1p_sb.rearrange("o (two g) -> o two g", two=2)
    wcv = wcp_sb.rearrange("o (two g) -> o two g", two=2)
    on8v = ones8.rearrange("o (two b) -> o two b", two=2)

    # ---- init loads ----
    # SP queue: Wg0+W8h0 first (gate the step-0 A pass), then wa0
    dma(Wg0[:], wg0)
    dma(W8h0[:], w8h0)
    dma(wa0_sb[:], wa0)
    # gpsimd queue: z tensors (f32r cast loads), states, xfT, then L1 weights
    wzT_sb = ldtmp.tile([ZD, 9], F32R, name="wzT_sb", tag="wz")
    gdma(wzT_sb[:], wzT)
    zT_sb = ldtmp.tile([ZD, P], F32R, name="zT_sb", tag="zt")
    gdma(zT_sb[:], zT)
    def half_states(pfx, dt):
        return [state.tile([P, H // 2], dt, name=f"{pfx}{u}", tag=f"{pfx}{u}")
                for u in (0, 1)]

    h0b_c = half_states("h0b", BF16)
    h08_c = half_states("h08", F8)
    h1b_c = half_states("h1b", BF16)
    h18_c = half_states("h18", F8)
    c0_c = half_states("c0", F16)
    c1_c = half_states("c1", F16)
    for u in (0, 1):
        gdma(h0b_c[u][:], h0b_d[u])
        gdma(h08_c[u][:], h08_d[u])
    xfT_sb = ldtmp.tile([EXO, S * P], BF16, name="xfT_sb", tag="xf")
    gdma(xfT_sb[:], xfT)
    for u in (0, 1):
        gdma(c0_c[u][:], c0_d[u])
        gdma(h1b_c[u][:], h1b_d[u])
        gdma(h18_c[u][:], h18_d[u])
        gdma(c1_c[u][:], c1_d[u])
    gdma(Wgh1[:], wgh1)
    gdma(W8h1[:], w8h1)
    gdma(Wgi1[:], wgi1)
    gdma(W8i1[:], w8i1)
    # ACT queue: dummy sigmoid first so the table set loads immediately,
    # then the small admas that gate the zb ACTs
    dumm = ldtmp.tile([1, 1], F32, name="dumm", tag="dumm")
    _reg("Memset", "dumm")
    nc.vector.memset(dumm[:], 0.0)
    _reg("Activation", "dumm")
    nc.scalar.activation(dumm[:], dumm[:], AF.Sigmoid)
    adma(bz8_sb[:], bz8)
    adma(bz0_sb[:], bz0)
    y0T_sb = ldtmp.tile([1, P], F32, name="y0T_sb", tag="y0")
    adma(y0T_sb[:], y0T)
    adma(inT[8:9, :], ones_row)  # b0 ones row (host const)
    adma(b1p_sb[:], b1p)
    adma(wcp_sb[:], wcp)

    _reg("Memset", "ones8")
    nc.vector.memset(ones8[:], 1.0)

    # ---- z bias: zb8[8, P] = W_z[1:9] @ z.T + b_z[1:9]; zby = row 0 ----
    zb_ps = g0p.tile([P, 2 * H], F32, name="zbps", tag="u0")
    _reg("Matmult", "zb")
    nc.tensor.matmul(
        zb_ps[0:8, 0:P], wzT_sb[:, 1:9], zT_sb[:],
        start=True, stop=True,
    )
    _reg("Matmult", "zb")
    nc.tensor.matmul(
        zb_ps[0:1, P : 2 * P], wzT_sb[:, 0:1], zT_sb[:],
        start=True, stop=True,
    )
    _reg("Activation", "zb8")
    nc.scalar.activation(zb8[:], zb_ps[0:8, 0:P], AF.Identity, bias=bz8_sb[:])
    _reg("Activation", "zby")
    nc.scalar.activation(zby[:], zb_ps[0:1, P : 2 * P], AF.Identity, bias=bz0_sb[:])

    # rest of the ACT-queue loads (after the zb ACTs so they don't gate them)
    bp_sb = ldtmp.tile([1, 1], F32, name="bp_sb", tag="bp")
    adma(bp_sb[:], bp)
    wpf = ldtmp.tile([P, KC], F32, name="wpf", tag="wp")
    adma(wpf[:], wpT)
    _reg("TensorCopy", "wpT")
    nc.vector.tensor_copy(wpT_sb[:], wpf[:])  # f32 -> bf16

    # exo rows with z-bias baked in: write straight into inT rows 0-7
    SPLIT = 4
    x3 = xfT_sb.rearrange("e (t b) -> e t b", b=P)
    dst3 = inT[0:8, :].rearrange("e (t b) -> e t b", b=P)
    zb3a = zb8.unsqueeze(1).broadcast_to((EXO, SPLIT, P))
    zb3b = zb8.unsqueeze(1).broadcast_to((EXO, S - SPLIT, P))
    _reg("TensorTensor", "exo_a")
    nc.vector.tensor_tensor(dst3[:, 0:SPLIT], x3[:, 0:SPLIT], zb3a, op=OP.add)
    # y0 row (partition 9, engine-unwritable): staged base-0 then DMA'd.
    # Only block 0 is ever read (t=0 uses K=10; t>0 uses K=9).
    fr0 = ldtmp.tile([1, P], BF16, name="fr0", tag="fr0")
    _reg("TensorTensor", "fr0")
    nc.vector.tensor_tensor(fr0[:], y0T_sb[:], zby[:], op=OP.add)
    adma(inT[9:10, 0:P], fr0[:])
    _reg("TensorCopy", "bp_row")
    nc.vector.tensor_copy(bp_row[:], bp_sb[0:1, 0:1].broadcast_to((1, P)))
    _reg("TensorTensor", "zbybp")
    nc.vector.tensor_tensor(zbybp[:], zby[:], bp_row[:], op=OP.add)
    _reg("TensorTensor", "exo_b")
    nc.vector.tensor_tensor(dst3[:, SPLIT:S], x3[:, SPLIT:S], zb3b, op=OP.add)

    # =========================== STEP LOOP ===========================
    CUR = ["init"]

    def R(role):
        CUR[0] = role

    def mm(*a, **k):
        _reg("Matmult", CUR[0])
        return _tag_role(nc.tensor.matmul(*a, **k), CUR[0])

    def sact(*a, **k):
        _reg("Activation", CUR[0])
        return _tag_role(nc.scalar.activation(*a, **k), CUR[0])

    def vtt(*a, **k):
        _reg("TensorTensor", CUR[0])
        return _tag_role(nc.vector.tensor_tensor(*a, **k), CUR[0])

    def g_tile(pool, u, pfx):
        return pool.tile([P, 2 * H], F32, name=pfx + f"u{u}", tag=f"u{u}")

    def hv(t4):
        """[P, KC*HB]-layout half state -> [P, KC, HB] view."""
        return t4.rearrange("p (k b) -> p k b", b=HB)

    def bkey(c):
        return "b0" if slot(c) < 8 else "b1"

    def dst_of(gt, c):
        s = slot(c)
        return gt[:, s * HB : (s + 1) * HB]

    def hid_mms(gt, c, W8v, Wg, h8u, hbu, started, stop=False):
        """Hidden-side matmuls for one gate chunk of one half: fp8 DR for
        i/f/o, bf16 for g. One PSUM start per bank (zero region)."""
        dst = dst_of(gt, c)
        bk = bkey(c)
        if c in G_CHUNKS:
            j0 = (c - 8) * P
            hb3 = hv(hbu)
            for k in range(KC):
                mm(
                    dst, Wg[:, k * H + j0 : k * H + j0 + P],
                    hb3[:, k : k + 1, :],
                    start=not started.get(bk, False),
                    stop=(stop and k == KC - 1),
                )
                started[bk] = True
        else:
            h83 = hv(h8u)
            for k2 in range(2):
                mm(
                    dst, W8v[:, 2 * k2 : 2 * k2 + 2, c * P : (c + 1) * P],
                    h83[:, 2 * k2 : 2 * k2 + 2, :],
                    start=not started.get(bk, False),
                    stop=(stop and k2 == 1),
                    perf_mode=DRM,
                )
                started[bk] = True

    def emit_A(gt, h08u, h0bu, started):
        R("A")
        for c in CH_ALL:
            hid_mms(gt, c, W8h0v, Wg0, h08u, h0bu, started)

    def emit_B(gt, t, u, parts=None):
        """L0 input matmuls: K=10 at t=0 (y0 row), K=9 after (exo + ones).
        With parts (t=0 only): carries the bank stops."""
        K = 10 if t == 0 else 9
        rhs = inT[0:K, t * P + u * HB : t * P + u * HB + HB]
        if parts is None:
            R("B")
            for c in CH_ALL:
                mm(dst_of(gt, c), wa0_sb[0:K, c * P : (c + 1) * P], rhs,
                   start=False, stop=False)
        else:
            R("B")
            for c in CH_B1:
                mm(dst_of(gt, c), wa0_sb[0:K, c * P : (c + 1) * P], rhs,
                   start=False, stop=(c == 11))
            parts[0]()
            R("B")
            for c in CH_B0:
                mm(dst_of(gt, c), wa0_sb[0:K, c * P : (c + 1) * P], rhs,
                   start=False, stop=(c == 7))
            parts[1]()  # sg only; caller runs the rest via run_tails

    def emit_feed(gt, s8v, parts):
        """y-feedback: one K=1 DR outer product per chunk against the fp8
        (hi, lo) wcol rows; carries the bank stops; tail parts interleave."""
        R("feed")
        for c in CH_B1:
            mm(dst_of(gt, c), wcv[:, :, c * P : (c + 1) * P], s8v,
               start=False, stop=(c == 11), perf_mode=DRM)
        parts[0]()
        R("feed")
        for c in CH_B0:
            mm(dst_of(gt, c), wcv[:, :, c * P : (c + 1) * P], s8v,
               start=False, stop=(c == 7), perf_mode=DRM)
        parts[1]()

    def emit_C(gt, h18u, h1bu, started):
        """L1 h1-part matmuls + fp8 DR bias (start-side)."""
        for c in CH_ALL:
            R("C")
            hid_mms(gt, c, W8h1v, Wgh1, h18u, h1bu, started)
            R("Cb")
            mm(dst_of(gt, c), b1v[:, :, c * P : (c + 1) * P],
               on8v[:], start=False, stop=False, perf_mode=DRM)

    def emit_D(gt, h08u, h0bu, parts):
        """L1 h0-part matmuls: the bank-closing stops; tail parts
        interleave between bank groups."""
        R("D")
        for c in CH_B1:
            hid_mms(gt, c, W8i1v, Wgi1, h08u, h0bu, {bkey(c): True},
                    stop=(c == 11))
        parts[0]()
        R("D")
        for c in CH_B0:
            hid_mms(gt, c, W8i1v, Wgi1, h08u, h0bu, {bkey(c): True},
                    stop=(c == 7))
        parts[1]()

    def make_tail(gt, c_prev_u, u, c_n_u, h_b_u, h_8_u, htag, h8_first):
        """Tail for one (layer, half) as 5 staged callbacks (tg, sg, c-chain,
        tanh(c), h-writes). The caller interleaves stages across the two
        halves so neither half's pending op blocks the other's ready op on
        the in-order ACT/DVE queues."""
        sg = act.tile([P, 3 * H // 2], F16, name=f"sg{u}", tag=f"sg{u}")
        tgt = act.tile([P, H // 2], F16, name=f"tg{u}", tag=f"tg{u}")
        t1 = act.tile([P, H // 2], F16, name=f"t1{u}", tag=f"t1{u}")
        t2 = act.tile([P, H // 2], F16, name=f"t2{u}", tag=f"t2{u}")
        thc = act.tile([P, H // 2], F16, name=f"thc{u}", tag=f"thc{u}")
        cpv = hv(c_prev_u)
        cnv = hv(c_n_u)
        hbv = hv(h_b_u)
        h8v = hv(h_8_u)
        sg3 = sg.rearrange("p (s b) -> p s b", b=HB)
        sgi, sgf, sgo = sg3[:, 0:4], sg3[:, 4:8], sg3[:, 8:12]
        tg3 = tgt.rearrange("p (s b) -> p s b", b=HB)
        t13 = t1.rearrange("p (s b) -> p s b", b=HB)
        t23 = t2.rearrange("p (s b) -> p s b", b=HB)
        th3 = thc.rearrange("p (s b) -> p s b", b=HB)
        ISC = 1.0 / SCL

        def p_g():
            R(htag + f":tg{u}")
            sact(tg3[:], gt[:, 12 * HB : 16 * HB].rearrange(
                "p (s b) -> p s b", b=HB), AF.Tanh, scale=ISC)

        def p_sg():
            R(htag + f":sg{u}")
            sact(sg[:], gt[:, 0 : 12 * HB], AF.Sigmoid, scale=ISC)

        def p_chain():
            R(htag + f":cchain{u}")
            vtt(t23[:], sgf, cpv, op=OP.mult)
            vtt(t13[:], sgi, tg3[:], op=OP.mult)
            vtt(cnv, t13[:], t23[:], op=OP.add)

        def p_thc():
            R(htag + f":thc{u}")
            sact(th3[:], cnv, AF.Tanh)

        def p_hwr():
            R(htag + f":hwr{u}")
            outs = [h8v, hbv] if h8_first else [hbv, h8v]
            for dst in outs:
                vtt(dst, sgo, th3[:], op=OP.mult)

        return [p_g, p_sg, p_chain, p_thc, p_hwr]

    # prologue: step-0 L0 gates for both halves (+tails via B stops), L1 start
    g0_cur = [g_tile(g0p, u, "g0") for u in (0, 1)]
    for u in (0, 1):
        emit_A(g0_cur[u], h08_c[u], h0b_c[u], {})
    c0_n = half_states("c0", F16)
    h0b_n = half_states("h0b", BF16)
    h08_n = half_states("h08", F8)
    g1_cur = [g_tile(g1p, u, "g1") for u in (0, 1)]
    for u in (0, 1):
        tail = make_tail(g0_cur[u], c0_c[u], u, c0_n[u], h0b_n[u], h08_n[u],
                         "h0", h8_first=True)
        emit_B(g0_cur[u], 0, u, tail)
        tail[2](); tail[3](); tail[4]()
        emit_C(g1_cur[u], h18_c[u], h1b_c[u], {})
    c0_c, h0b_cur, h08_cur = c0_n, h0b_n, h08_n

    for t in range(S):
        # ---- per half: close L1(t) with D + full tail, then start L0(t+1)
        c1_n = half_states("c1", F16)
        h1b_n = half_states("h1b", BF16)
        h18_n = half_states("h18", F8)
        if t + 1 < S:
            g0_nxt = [g_tile(g0p, u, "g0") for u in (0, 1)]
        for u in (0, 1):
            tail = make_tail(g1_cur[u], c1_c[u], u, c1_n[u], h1b_n[u],
                             h18_n[u], "h1", h8_first=False)
            emit_D(g1_cur[u], h08_cur[u], h0b_cur[u], tail)
            tail[2](); tail[3](); tail[4]()
            if t + 1 < S:
                emit_A(g0_nxt[u], h08_cur[u], h0b_cur[u], {})
                emit_B(g0_nxt[u], t + 1, u)
        c1_c = c1_n
        # ---- per half: proj/y/s8, then close L0(t+1) with feed + full tail,
        # then start L1(t+1)
        if t + 1 < S:
            c0_n = half_states("c0", F16)
            h0b_n = half_states("h0b", BF16)
            h08_n = half_states("h08", F8)
            g1_nxt = [g_tile(g1p, u, "g1") for u in (0, 1)]
        for u in (0, 1):
            h1b3 = hv(h1b_n[u])
            u0, u1 = u * HB, u * HB + HB
            pj = g1_cur[u][0:1, 11 * HB : 12 * HB]
            R("proj")
            for k in range(KC):
                mm(pj, wpT_sb[:, k : k + 1], h1b3[:, k : k + 1, :],
                   start=(k == 0), stop=(k == KC - 1))
            s8 = yo.tile([1, 2 * HB], F8, name=f"s8{u}", tag=f"s8{u}")
            s8v = s8.rearrange("o (two b) -> o two b", two=2)
            R("s8")
            vtt(s8v[:],
                pj.unsqueeze(1).broadcast_to((1, 2, HB)),
                zbybp[:, u0:u1].unsqueeze(1).broadcast_to((1, 2, HB)),
                op=OP.add)
            yo_t = yo.tile([1, HB], F32, name=f"yout{u}", tag=f"yout{u}")
            R("y_out")
            vtt(yo_t[:], pj, bp_row[:, u0:u1], op=OP.add)
            dma(out_d[t : t + 1, u0:u1], yo_t[:])
            if t + 1 < S:
                tail = make_tail(g0_nxt[u], c0_c[u], u, c0_n[u], h0b_n[u],
                                 h08_n[u], "h0", h8_first=True)
                emit_feed(g0_nxt[u], s8v, tail)
                tail[2](); tail[3](); tail[4]()
                emit_C(g1_nxt[u], h18_n[u], h1b_n[u], {})
        if t + 1 < S:
            c0_c, h0b_cur, h08_cur = c0_n, h0b_n, h08_n
            g0_cur, g1_cur = g0_nxt, g1_nxt


def make_nc(steps: int = S):
    nc = bacc.Bacc("TRN2", target_bir_lowering=False, debug=False)
    build_kernel(nc, steps)
    nc.compile()
    return nc


# ======================= host-side prep =======================

def _bf16(x):
    import ml_dtypes
    return np.ascontiguousarray(np.asarray(x, np.float32).astype(ml_dtypes.bfloat16))


def _f8(x):
    import ml_dtypes
    return np.ascontiguousarray(np.asarray(x, np.float32).astype(ml_dtypes.float8_e4m3fn))


def _f8pair(v):
    """f32 vector -> [1, 2*len] fp8 (hi row, lo row) of 16*v."""
    import ml_dtypes
    v = np.asarray(v, np.float32) * SCL
    hi = v.astype(ml_dtypes.float8_e4m3fn)
    lo = (v - hi.astype(np.float32)).astype(ml_dtypes.float8_e4m3fn)
    return np.ascontiguousarray(np.stack([hi, lo]).reshape(1, 2 * v.size))


def _f32(x):
    return np.ascontiguousarray(np.asarray(x, dtype=np.float32))


def _wT(W):
    """[G', H] f32 -> [128, KC*G'] with [p, k*G'+g] = W[g, k*128+p]."""
    W = np.asarray(W, dtype=np.float32)
    Gp = W.shape[0]
    return W.T.reshape(KC, P, Gp).transpose(1, 0, 2).reshape(P, KC * Gp)


def _hT4(h):
    """[B_loc, H] -> [128, H] with [p, k*128+b] = h[b, k*128+p]."""
    return h.T.reshape(KC, P, P).transpose(1, 0, 2).reshape(P, H)


def shard_inputs(inputs, steps: int):
    B = inputs["y0"].shape[0]
    nb = B // P
    Wih0 = np.asarray(inputs["W_ih0"], np.float32)
    Wp = np.asarray(inputs["W_proj"], np.float32)
    b0 = _f32(inputs["b_ih0"]) + _f32(inputs["b_hh0"])
    b1 = _f32(inputs["b_ih1"]) + _f32(inputs["b_hh1"])
    wa0 = np.concatenate(
        [
            Wih0[:, 1:9].T * SCL,          # exo rows
            b0.reshape(1, G) * SCL,        # ones-row bias
            Wih0[:, 0:1].T * SCL,          # y0 row (t=0 only)
        ],
        axis=0,
    )

    def wsplit(W):
        W = np.asarray(W, np.float32) * SCL
        return _f8(_wT(W)), _bf16(_wT(W[1024:1536]))

    w8h0, wg0 = wsplit(inputs["W_hh0"])
    w8i1, wgi1 = wsplit(inputs["W_ih1"])
    w8h1, wgh1 = wsplit(inputs["W_hh1"])
    rep = {
        "w8h0": w8h0, "wg0": wg0,
        "w8i1": w8i1, "wgi1": wgi1,
        "w8h1": w8h1, "wgh1": wgh1,
        "wa0": _bf16(wa0),
        "b1p": _f8pair(b1),
        "wcp": _f8pair(Wih0[:, 0]),
        "wpT": _f32(Wp).reshape(KC, P).T.copy(),
        "bp": _f32(inputs["b_proj"]).reshape(1, 1),
        "wzT": _f32(inputs["W_z"]).T.copy(),
        "bz8": _f32(inputs["b_z"]).reshape(9, 1)[1:9].copy(),
        "bz0": _f32(inputs["b_z"]).reshape(9, 1)[0:1].copy(),
        "ones_row": _bf16(np.ones((1, S * P), np.float32)),
    }
    maps = []
    for i in range(nb):
        s = slice(i * P, (i + 1) * P)
        x = np.asarray(inputs["x_future"], np.float32)[s, :steps]  # [P, S, E]
        h0b = _bf16(_hT4(np.asarray(inputs["h0"], np.float32)[0, s]))
        h1b = _bf16(_hT4(np.asarray(inputs["h0"], np.float32)[1, s]))
        c0h = _hT4(np.asarray(inputs["c0"], np.float32)[0, s]).astype(np.float16)
        c1h = _hT4(np.asarray(inputs["c0"], np.float32)[1, s]).astype(np.float16)

        def hsplit(full, u):
            """[P, 512] = [p, k*128+b] -> half [P, 256] = [p, k*64+b]"""
            v = np.asarray(full).reshape(P, KC, P)[:, :, u * HB : (u + 1) * HB]
            return np.ascontiguousarray(v.reshape(P, KC * HB))

        m = dict(rep)
        m.update(
            {
                "zT": _f32(inputs["z"][s]).T.copy(),
                "y0T": _f32(inputs["y0"][s]).reshape(1, P).copy(),
                "xfT": _bf16(x.transpose(2, 1, 0).reshape(EXO, steps * P)),
            }
        )
        for u in (0, 1):
            m[f"h0b{u}"] = hsplit(h0b, u)
            m[f"h1b{u}"] = hsplit(h1b, u)
            m[f"h08{u}"] = hsplit(_f8(h0b.astype(np.float32)), u)
            m[f"h18{u}"] = hsplit(_f8(h1b.astype(np.float32)), u)
            m[f"c0T{u}"] = hsplit(c0h, u)
            m[f"c1T{u}"] = hsplit(c1h, u)
        maps.append(m)
    return maps


def assemble_output(results, steps: int):
    outs = [
        np.ascontiguousarray(np.asarray(rm["out"]).T).reshape(P, steps, 1)
        for rm in results
    ]
    return np.concatenate(outs, axis=0)


# ======================= public entry point =======================
_NC_CACHE = {}


def _get_nc():
    if "nc" not in _NC_CACHE:
        _NC_CACHE["nc"] = make_nc(S)
    return _NC_CACHE["nc"]


STEPS = S
N_CORES = 8


def kernel(**inputs):
    """Full-input entry point: shards batch over 8 NeuronCores, runs the
    Bass LSTM-decoder kernel, reassembles [B, steps, 1] float32 output."""
    from concourse.bass_utils import run_bass_kernel_spmd

    steps = int(inputs.get("steps", STEPS))
    assert steps == STEPS, f"kernel compiled for {STEPS} steps, got {steps}"
    nc = _get_nc()
    maps = shard_inputs(inputs, STEPS)
    res = run_bass_kernel_spmd(nc, maps, list(range(N_CORES)))
    return assemble_output(res.results, STEPS).astype(np.float32)
